# revision 64
# baseline (speedup 1.0000x reference)
"""Trainium2 Bass kernel for nn_DiscreteModel (GNN message passing).

Strategy: shard by node rows across 8 cores (512 rows each). All per-node
tensors are kept feature-major ([feature, node]) on-chip so the contraction
dim of every matmul sits on SBUF partitions. The host pre-transposes the
od_mat shard and all weights, folds the random-walk projection W_rw and the
1/8 mean into the layer-1 weight block, and pads HID 2112 -> 2176.

v3 (~125us, from the 158us v2). The PE issues a warm fp8-DoubleRow matmul
every ~216ns (2.4GHz; the 380ns trace "duration" is issue-to-drain), so the
od x W1 block floors at ~59us and everything else must hide behind it:
  head   : od loaded as 8 chunk tiles (2KB partition lines; per-chunk
           matmul gating, odc1 queued ahead of w1tB0 -> first DR matmul
           ~8.4us); w1 h-tiles split in halves across the sync/scalar
           queues; the w1m/w2t constant loads are split across slots
           5/6/7 so their backlog stays under the per-slot DMA slack;
           4 zero-DR warmup matmuls fill the PE until od chunk 0 lands.
  gather : 32 indirect DMAs (~1.1us of gpsimd SWDGE ucode each, ends
           ~56us; dma_gather would pay a ~13us Q7 ucode lib load first).
           gpsimd carries NOTHING else until the output DMAs.
  spill  : h<9 evacuate their od partial to SBUF (scalar ACT) and replay
           after the mixed k-tile exists (lag-2 finalize for h>=9, one
           replay per slot from slot 11, short drain; transposes emitted
           at slot 10, after the walk sums land ~60us). The walk-sum
           transposes read identx = ident + 0*odp[8] -- a data-dep pin so
           the Tile scheduler (sim-driven, reorders freely) cannot hoist
           them and their semaphore stalls into the early od stream.
  L2     : col-tiled pairs (even h -> psum[0:64], odd h -> [64:128],
           concurrent); halves combined for free via duplicated GRU Wi
           rows (gi = [Wi;Wi] @ [msgA;msgB], K=128 costs the same).
  relu   : alternates scalar ACT / vector add+max so neither engine paces
           the finalize chain; replay adds on vector, replay relu scalar.
  tail   : GRU memory operand in bf16 from the resident memT tile; wp1
           output M=128 with [Wp1|Wp1] so act lands duplicated on
           partitions 0:64/64:128; pred runs as 16 row-tiled PAIRS
           (tile_position (0,0)/(64,0), concurrent); evacuations alternate
           scalar/vector into bf16 staging; output DMA per 4 m-tiles on
           sync/gpsimd with [128, 32, R] DRAM layout (4KB lines).
fp8    : the od x W1 block (K=4096 of 4224) runs in fp8e4 DoubleRow mode.
         W1od is scaled x16 on host (relu(16x)=16relu(x); 1/16 folded
         into W2). Rel err ~1.24e-2 vs the 2e-2 gate.
Note: the axon TRN2 fleet drifts run-to-run (same NEFF 124..151us);
compare kernels only back-to-back within one window, min-of-3.
"""

import numpy as np

import concourse.bass as bass
import concourse.bacc as bacc
import concourse.tile as tile
from concourse import mybir

N = 4096        # nodes
MD = 64         # memory dim
MSG = 64        # message dim
WL = 8          # walk length
HID = 2112
HT = 17         # h-tiles (HID padded to 17*128 = 2176)
HIDP = HT * 128
NC = 8          # cores
R = N // NC     # rows (nodes) per core = 512
NT = R // 128   # node tiles per core = 4
F32 = mybir.dt.float32
F32R = mybir.dt.float32r
BF16 = mybir.dt.bfloat16
FP8 = mybir.dt.float8e4
I16 = mybir.dt.int16
I32 = mybir.dt.int32
WSCALE = 16.0   # W1 block scale so fp8 weights sit in e4m3 normal range
USE_DMA_GATHER = False   # Q7 SWDGE gather pays ~13us ucode lib load; the 32
                         # indirect DMAs (1.1us gpsimd ucode each) end sooner
N_WARM = 4      # zero DR matmuls fill the PE only until the first od
                # chunk lands (~8.7us); more would delay the real stream
LAG = 2         # h-tiles between od part and mixed finalize

_PROG = None


def _build_program():
    nc = bacc.Bacc("TRN2", target_bir_lowering=False, debug=False, num_devices=NC)

    # ---- DRAM I/O (all pre-laid-out on host, partition-major) ----
    mem_d = nc.dram_tensor("mem", [N, MD], F32, kind="ExternalInput").ap()
    memT_d = nc.dram_tensor("memT", [MD, R], BF16, kind="ExternalInput").ap()
    od_d = nc.dram_tensor("odv", [128, 32 * R], FP8, kind="ExternalInput").ap()
    if USE_DMA_GATHER:
        widx_d = nc.dram_tensor("widx", [128, NT * WL * 128 // 16], I16,
                                kind="ExternalInput").ap()
    else:
        widx32_d = nc.dram_tensor("widx", [128, NT * WL], I32,
                                  kind="ExternalInput").ap()
    w1h_d = nc.dram_tensor("w1h", [HT, 128, 32 * 128], FP8, kind="ExternalInput").ap()
    w1m_d = nc.dram_tensor("w1m", [128, HT * 128], BF16, kind="ExternalInput").ap()
    w2t_d = nc.dram_tensor("w2t", [128, HT * MSG], BF16, kind="ExternalInput").ap()
    wi2_d = nc.dram_tensor("wi2", [128, 3 * MD], BF16, kind="ExternalInput").ap()
    wh_d = nc.dram_tensor("wh", [MD, 3 * MD], BF16, kind="ExternalInput").ap()
    wp1_d = nc.dram_tensor("wp1x", [MD, 128], BF16, kind="ExternalInput").ap()
    wp2_d = nc.dram_tensor("wp2d", [128, N], BF16, kind="ExternalInput").ap()
    bias_d = nc.dram_tensor("biases", [128, 64], F32, kind="ExternalInput").ap()
    ident_d = nc.dram_tensor("ident", [128, 128], F32, kind="ExternalInput").ap()
    out_d = nc.dram_tensor("outm", [128, 32, R], BF16, kind="ExternalOutput").ap()

    AF = mybir.ActivationFunctionType
    DR = mybir.MatmulPerfMode.DoubleRow
    HK = 8 * 256            # half of a w1 h-tile (k-pairs 0..7)

    with tile.TileContext(nc) as tc:
        with (
            tc.tile_pool(name="consts", bufs=1) as consts,
            tc.tile_pool(name="w1p", bufs=3) as w1p,
            tc.tile_pool(name="gp", bufs=2) as gp,
            tc.tile_pool(name="hp", bufs=4) as hp,
            tc.tile_pool(name="gates", bufs=1) as gates,
            tc.tile_pool(name="ostg", bufs=3) as ostg,
            tc.tile_pool(name="pmm", bufs=5, space="PSUM") as pmm,
            tc.tile_pool(name="pacc", bufs=1, space="PSUM") as pacc,
            tc.tile_pool(name="ptr", bufs=1, space="PSUM") as ptr,
        ):
            # ---- walk indices first: gather feeds the mixed k-tile.
            # Split per node-tile so the first indirect DMA starts as soon
            # as its own 4KB of indices lands.
            if USE_DMA_GATHER:
                wk = consts.tile([128, NT * WL * 128 // 16], I16, tag="wk")
                nc.gpsimd.dma_start(out=wk[:], in_=widx_d[:])
            else:
                wk = consts.tile([128, NT * WL], I32, tag="wk")
                for t in range(NT):
                    nc.gpsimd.dma_start(out=wk[:, t * WL:(t + 1) * WL],
                                        in_=widx32_d[:, t * WL:(t + 1) * WL])

            # head DMA: the first DR matmul needs w1 h0 front half + od chunk
            # 0 only -> both lead their queues; od goes in 8 chunks of 2
            # k-pairs (2KB partition lines for full DMA efficiency), even on
            # sync, odd on scalar, so matmul k gates on chunk k//2.
            w1tA = [None] * HT
            w1tB = [None] * HT
            w1tA[0] = w1p.tile([128, HK], FP8, tag="w1tA", name="w1tA0")
            nc.sync.dma_start(out=w1tA[0][:], in_=w1h_d[0][:, :HK])
            odc = []
            for c in range(8):
                t = consts.tile([128, 4 * R], FP8, tag=f"odc{c}",
                                name=f"odc{c}")
                odc.append(t)
            nc.scalar.dma_start(out=odc[0][:], in_=od_d[:, 0:4 * R])
            nc.scalar.dma_start(out=odc[1][:], in_=od_d[:, 4 * R:8 * R])
            w1tB[0] = w1p.tile([128, HK], FP8, tag="w1tB", name="w1tB0")
            nc.scalar.dma_start(out=w1tB[0][:], in_=w1h_d[0][:, HK:])
            for c in range(2, 8):
                eng = nc.sync if c % 2 == 0 else nc.scalar
                eng.dma_start(out=odc[c][:],
                              in_=od_d[:, c * 4 * R:(c + 1) * 4 * R])
            for h in (1, 2):
                w1tA[h] = w1p.tile([128, HK], FP8, tag="w1tA", name=f"w1tA{h}")
                nc.sync.dma_start(out=w1tA[h][:], in_=w1h_d[h][:, :HK])
                w1tB[h] = w1p.tile([128, HK], FP8, tag="w1tB", name=f"w1tB{h}")
                nc.scalar.dma_start(out=w1tB[h][:], in_=w1h_d[h][:, HK:])

            # Q7 SWDGE gather for all 4096 walk rows:
            # gare[p, (t*WL+j)*MD : +MD] = mem[walks[t*128+p, j]]
            gare = consts.tile([128, NT * WL * MD], F32, tag="gare")
            if USE_DMA_GATHER:
                # >=2048 idxs per instruction hangs the Q7 ucode on HW;
                # 4x1024 (one per node-tile) costs ~1.2us SWDGE each
                for t in range(NT):
                    nc.gpsimd.dma_gather(
                        gare[:, t * WL * MD:(t + 1) * WL * MD].rearrange(
                            "p (g d) -> p g d", g=WL),
                        mem_d[:], wk[:, t * WL * 8:(t + 1) * WL * 8],
                        WL * 128, WL * 128, MD,
                    )
            else:
                for t in range(NT):
                    for j in range(WL):
                        o = (t * WL + j) * MD
                        nc.gpsimd.indirect_dma_start(
                            out=gare[:, o:o + MD],
                            out_offset=None,
                            in_=mem_d[:],
                            in_offset=bass.IndirectOffsetOnAxis(
                                ap=wk[:, t * WL + j:t * WL + j + 1], axis=0),
                        )

            # PE warmup: zero DR matmuls from ~6.5us pull HAM to full clock
            # before the real stream starts (idle >3.4us re-throttles)
            zx = consts.tile([128, 2 * R], FP8, tag="zx")
            nc.vector.memset(zx[:], 0)
            pdum = pmm.tile([128, R], F32, tag="mm")
            for _ in range(N_WARM):
                nc.tensor.matmul(
                    out=pdum[:],
                    lhsT=zx[:, 0:256].rearrange("p (two m) -> p two m", two=2),
                    rhs=zx[:].rearrange("p (two n) -> p two n", two=2),
                    start=True, stop=True, perf_mode=DR,
                )

            # identity from DRAM: gpsimd must stay free for the gather ucode
            ident = consts.tile([128, 128], F32, tag="ident")
            nc.scalar.dma_start(out=ident[:], in_=ident_d[:])
            biasp = consts.tile([128, 64], F32, tag="biasp")
            nc.scalar.dma_start(out=biasp[:], in_=bias_d[:])

            # mixed rawT k-tile: [0:64] = memT shard, [64:128] = GsT (walk sums)
            mixed = consts.tile([128, R], BF16, tag="mixed")
            nc.scalar.dma_start(out=mixed[0:MD, :], in_=memT_d[:])

            # constant tiles; their DMAs are emitted inside the h-loop so
            # they queue behind the od/w1 head flood (needed ~55us onward)
            w1m_sb = consts.tile([128, HT * 128], BF16, tag="w1m")
            w2t_sb = consts.tile([128, HT * MSG], BF16, tag="w2t")
            wh_sb = consts.tile([MD, 3 * MD], BF16, tag="wh")
            wi2_sb = consts.tile([128, 3 * MD], BF16, tag="wi2")
            wp1_sb = consts.tile([MD, 128], BF16, tag="wp1x")
            wp2_sb = consts.tile([128, N], BF16, tag="wp2d")

            # preload the sigmoid/tanh ACT table while the head is DMA-paced
            # (otherwise a 1.28us ACT_TABLE_LOAD lands on the GRU chain)
            warm = gates.tile([MD, 4], F32, tag="warm")
            nc.scalar.activation(warm[:, 0:2], biasp[0:MD, 0:2], AF.Sigmoid)
            nc.scalar.activation(warm[:, 2:4], biasp[0:MD, 0:2], AF.Tanh)

            # walk sums on DVE (gated on the gather), one per node-tile
            m1s = [None] * NT
            for t in range(NT):
                ga3 = gare[:, t * WL * MD:(t + 1) * WL * MD].rearrange(
                    "p (j d) -> p j d", j=WL)
                m4 = gp.tile([128, 4 * MD], F32, tag="m4")
                m43 = m4[:].rearrange("p (j d) -> p j d", j=4)
                nc.vector.tensor_add(out=m43, in0=ga3[:, 0:4, :], in1=ga3[:, 4:8, :])
                m2 = gp.tile([128, 2 * MD], F32, tag="m2")
                m23 = m2[:].rearrange("p (j d) -> p j d", j=2)
                nc.vector.tensor_add(out=m23, in0=m43[:, 0:2, :], in1=m43[:, 2:4, :])
                m1t = gp.tile([128, MD], F32, tag=f"m1_{t}")
                nc.vector.tensor_add(out=m1t[:], in0=m2[:, 0:MD],
                                     in1=m2[:, MD:2 * MD])
                m1s[t] = m1t

            mixed_r = mixed[:]

            # ---- layer 1 (fp8 DoubleRow); the gather (4x ~8.6us Q7 ucode)
            # only completes ~44us in, so h < SPILL spill their od partial to
            # SBUF (freeing the PSUM bank) and replay one per slot once the
            # mixed tile exists; h >= SPILL run a lag-2 finalize.
            SPILL = 9
            psL2 = pacc.tile([128, R], F32, tag="l2")
            odp = consts.tile([128, SPILL * R], F32, tag="odp")
            identx = consts.tile([128, 128], F32, tag="identx")
            pss = {}
            hids = {}
            l2n = [0, 0]
            L2N = [9, 8]   # even/odd L2 stream lengths

            def emit_l2(h):
                half = h % 2
                nc.tensor.matmul(
                    out=psL2[half * 64:(half + 1) * 64, :],
                    lhsT=w2t_sb[:, h * MSG:(h + 1) * MSG],
                    rhs=hids.pop(h)[:],
                    start=(l2n[half] == 0), stop=(l2n[half] == L2N[half] - 1),
                )
                l2n[half] += 1

            def emit_relu(h, src):
                # alternate relu between scalar ACT and vector (add-bias,
                # max 0) so neither engine paces the finalize/replay chain
                hid = hp.tile([128, R], BF16, tag="hid")
                if h % 2 == 0:
                    nc.scalar.activation(hid[:], src, AF.Relu,
                                         bias=biasp[:, h:h + 1])
                else:
                    nc.vector.tensor_scalar(
                        out=hid[:], in0=src, scalar1=biasp[:, h:h + 1],
                        scalar2=0.0, op0=mybir.AluOpType.add,
                        op1=mybir.AluOpType.max)
                hids[h] = hid

            def finalize(h):
                ps = pss.pop(h)
                nc.tensor.matmul(
                    out=ps[:], lhsT=w1m_sb[:, h * 128:(h + 1) * 128],
                    rhs=mixed_r, start=False, stop=True,
                )
                emit_relu(h, ps[:])
                emit_l2(h)

            def replay(h):
                ps = pmm.tile([128, R], F32, tag="mm")
                nc.tensor.matmul(
                    out=ps[:], lhsT=w1m_sb[:, h * 128:(h + 1) * 128],
                    rhs=mixed_r, start=True, stop=True,
                )
                pre = gp.tile([128, R], F32, tag="clt")
                nc.vector.tensor_add(out=pre[:], in0=ps[:],
                                     in1=odp[:, h * R:(h + 1) * R])
                # vector already carries the add: replay relu goes to scalar
                hid = hp.tile([128, R], BF16, tag="hid")
                nc.scalar.activation(hid[:], pre[:], AF.Relu,
                                     bias=biasp[:, h:h + 1])
                hids[h] = hid
                emit_l2(h)

            for h in range(HT):
                if h >= 3:
                    w1tA[h] = w1p.tile([128, HK], FP8, tag="w1tA",
                                       name=f"w1tA{h}")
                    w1tB[h] = w1p.tile([128, HK], FP8, tag="w1tB",
                                       name=f"w1tB{h}")
                    engA = nc.sync if h % 2 == 1 else nc.scalar
                    engB = nc.scalar if h % 2 == 1 else nc.sync
                    engA.dma_start(out=w1tA[h][:], in_=w1h_d[h][:, :HK])
                    engB.dma_start(out=w1tB[h][:], in_=w1h_d[h][:, HK:])
                ps = pmm.tile([128, R], F32, tag="mm")
                for k in range(16):
                    wt = w1tA[h] if k < 8 else w1tB[h]
                    nc.tensor.matmul(
                        out=ps[:],
                        lhsT=wt[:, (k % 8) * 256:(k % 8 + 1) * 256].rearrange(
                            "p (two m) -> p two m", two=2),
                        rhs=odc[k // 2][:, (k % 2) * 2 * R:(k % 2 + 1) * 2 * R]
                        .rearrange("p (two n) -> p two n", two=2),
                        start=(k == 0), stop=(h < SPILL),
                        perf_mode=DR,
                    )
                if h < SPILL:
                    # evacuate pre-activation od partial (scalar; the vector
                    # stream is held by the gather-gated walk sums)
                    nc.scalar.activation(odp[:, h * R:(h + 1) * R], ps[:],
                                         AF.Identity)
                else:
                    pss[h] = ps
                if h == SPILL - 1:
                    # identx = ident + 0*odp[h]: a scheduler pin — the mix
                    # transposes read identx, so no schedule can hoist them
                    # (and their semaphore stalls) into the early od stream
                    tmpid = gp.tile([128, 128], F32, tag="tmpid")
                    nc.vector.tensor_scalar_mul(
                        out=tmpid[:], in0=odp[:, h * R:h * R + 128],
                        scalar1=0.0)
                    nc.vector.tensor_add(out=identx[:], in0=ident[:],
                                         in1=tmpid[:])
                if h == 5:
                    # replays (h<9) read the low half first
                    nc.scalar.dma_start(out=w1m_sb[:, :9 * 128],
                                        in_=w1m_d[:, :9 * 128])
                if h == 6:
                    nc.sync.dma_start(out=w2t_sb[:], in_=w2t_d[:])
                if h == 7:
                    nc.scalar.dma_start(out=w1m_sb[:, 9 * 128:],
                                        in_=w1m_d[:, 9 * 128:])
                if h == 12:
                    # tail-only constants: late so they never delay the w1
                    # stream that gates the od matmuls
                    nc.scalar.dma_start(out=wh_sb[:], in_=wh_d[:])
                    nc.scalar.dma_start(out=wi2_sb[:], in_=wi2_d[:])
                    nc.scalar.dma_start(out=wp1_sb[:], in_=wp1_d[:])
                    nc.sync.dma_start(out=wp2_sb[:], in_=wp2_d[:])
                if h == 14:
                    # h_n = memT @ Wh_n + bias depends only on memT; do it in
                    # the ramp where the PE has slack
                    ps_hn = pmm.tile([MD, R], F32, tag="mm")
                    nc.tensor.matmul(out=ps_hn[:], lhsT=wh_sb[:, 128:192],
                                     rhs=mixed[0:MD, :], start=True, stop=True)
                    hnb = gates.tile([MD, R], F32, tag="hnb")
                    nc.vector.tensor_scalar_add(out=hnb[:], in0=ps_hn[:],
                                                scalar1=biasp[0:MD, 20:21])
                if h == SPILL + 1:
                    # GsT transposes; PE reaches them ~62us, sums done ~60us
                    for t in range(NT):
                        tr = ptr.tile([MD, 128], F32, tag="tr")
                        nc.tensor.transpose(out=tr[:], in_=m1s[t][:],
                                            identity=identx[:])
                        nc.vector.tensor_copy(
                            out=mixed[MD:128, t * 128:(t + 1) * 128], in_=tr[:])
                if h >= SPILL + LAG:
                    finalize(h - LAG)
                    replay(h - SPILL - LAG)
            finalize(HT - 2)
            finalize(HT - 1)
            for h in range(min(HT - SPILL - LAG, SPILL), SPILL):
                replay(h)

            # msg (pre-b2, which is folded into the GRU input bias):
            # both L2 col halves evacuated in one op; the GRU adds them by
            # using duplicated Wi rows (K=128 costs the same as K=64)
            msg2x = gates.tile([128, R], BF16, tag="msg2x")
            nc.scalar.activation(msg2x[:], psL2[:], AF.Identity)
            msg_r = msg2x[:]
            memT_r = mixed[0:MD, :]

            # ---- GRU + prediction, column-split so the serial ACT/DVE chain
            #      pipelines across halves and the PE never idles >3.4us.
            ps_r = pmm.tile([MD, R], F32, tag="mm")
            nc.tensor.matmul(out=ps_r[:], lhsT=wi2_sb[:, 0:MD], rhs=msg_r,
                             start=True, stop=False)
            nc.tensor.matmul(out=ps_r[:], lhsT=wh_sb[:, 0:MD], rhs=memT_r,
                             start=False, stop=True)
            ps_z = pmm.tile([MD, R], F32, tag="mm")
            nc.tensor.matmul(out=ps_z[:], lhsT=wi2_sb[:, MD:128], rhs=msg_r,
                             start=True, stop=False)
            nc.tensor.matmul(out=ps_z[:], lhsT=wh_sb[:, MD:128], rhs=memT_r,
                             start=False, stop=True)
            ps_in = pmm.tile([MD, R], F32, tag="mm")
            nc.tensor.matmul(out=ps_in[:], lhsT=wi2_sb[:, 128:192], rhs=msg_r,
                             start=True, stop=True)
            r_t = gates.tile([MD, R], F32, tag="r_t")
            z_t = gates.tile([MD, R], F32, tag="z_t")
            rhn = gates.tile([MD, R], F32, tag="rhn")
            npre = gates.tile([MD, R], F32, tag="npre")
            n_t = gates.tile([MD, R], F32, tag="n_t")
            zc_t = gates.tile([MD, R], F32, tag="zc_t")
            zm_t = gates.tile([MD, R], BF16, tag="zm_t")
            ncz = gates.tile([MD, R], BF16, tag="ncz")
            ps_pred = pacc.tile([128, R], F32, tag="pred")
            act2 = gates.tile([128, R], BF16, tag="act2")
            HR = R // 2
            for x in range(2):
                cs = slice(x * HR, (x + 1) * HR)
                nc.scalar.activation(r_t[:, cs], ps_r[:, cs], AF.Sigmoid,
                                     bias=biasp[0:MD, 17:18])
                nc.scalar.activation(z_t[:, cs], ps_z[:, cs], AF.Sigmoid,
                                     bias=biasp[0:MD, 18:19])
                # upd = (1-z)*n + z*mem = zc*n + zm; zc/zm go on vector right
                # after the z sigmoid (gpsimd would pay a Q7 ucode lib swap
                # after the gathers that serializes the whole chain)
                nc.vector.tensor_scalar(out=zc_t[:, cs], in0=z_t[:, cs],
                                        scalar1=-1.0, scalar2=1.0,
                                        op0=mybir.AluOpType.mult,
                                        op1=mybir.AluOpType.add)
                nc.vector.tensor_mul(out=zm_t[:, cs], in0=z_t[:, cs],
                                     in1=memT_r[:, cs])
                # upd = ncz + zm is absorbed into wp1 by linearity:
                # Wp1@(ncz+zm) = Wp1@ncz + Wp1@zm. The zm part runs right
                # after the z sigmoid, off the r->tanh critical chain
                nc.tensor.matmul(out=ps_pred[:, cs], lhsT=wp1_sb[:],
                                 rhs=zm_t[:, cs], start=True, stop=False)
                nc.vector.tensor_mul(out=rhn[:, cs], in0=r_t[:, cs], in1=hnb[:, cs])
                nc.vector.tensor_add(out=npre[:, cs], in0=ps_in[:, cs], in1=rhn[:, cs])
                nc.scalar.activation(n_t[:, cs], npre[:, cs], AF.Tanh,
                                     bias=biasp[0:MD, 19:20])
                nc.vector.tensor_mul(out=ncz[:, cs], in0=zc_t[:, cs], in1=n_t[:, cs])
                # [Wp1|Wp1] -> act duplicated on partitions 0:64 / 64:128 so
                # pred pairs can row-tile
                nc.tensor.matmul(out=ps_pred[:, cs], lhsT=wp1_sb[:], rhs=ncz[:, cs],
                                 start=False, stop=True)
                nc.scalar.activation(act2[:, cs], ps_pred[:, cs], AF.Relu,
                                     bias=biasp[:, 21:22])

            # ---- prediction m-loop: 16 row-tiled PAIRS (tile_position
            #      (0,0)/(64,0), concurrent on the PE); evacuations alternate
            #      scalar/vector; output staged bf16, 0.5MB DMAs
            GRP = 4
            for m2 in range(16):
                m0, m1 = 2 * m2, 2 * m2 + 1
                psA = pmm.tile([128, R], F32, tag="mm")
                psB = pmm.tile([128, R], F32, tag="mm")
                nc.tensor.matmul(out=psA[:],
                                 lhsT=wp2_sb[0:64, m0 * 128:(m0 + 1) * 128],
                                 rhs=act2[0:64, :], start=True, stop=True)
                nc.tensor.matmul(out=psB[:],
                                 lhsT=wp2_sb[64:128, m1 * 128:(m1 + 1) * 128],
                                 rhs=act2[64:128, :], start=True, stop=True)
                if m0 % GRP == 0:
                    stage = ostg.tile([128, GRP * R], BF16, tag="stage")
                slA = stage[:, (m0 % GRP) * R:(m0 % GRP + 1) * R]
                slB = stage[:, (m1 % GRP) * R:(m1 % GRP + 1) * R]
                nc.scalar.activation(slA, psA[:], AF.Identity,
                                     bias=biasp[:, 22 + m0:23 + m0])
                nc.vector.tensor_scalar_add(out=slB, in0=psB[:],
                                            scalar1=biasp[:, 22 + m1:23 + m1])
                if m1 % GRP == GRP - 1:
                    g = m1 // GRP
                    oeng = nc.sync if g % 2 == 0 else nc.gpsimd
                    oeng.dma_start(
                        out=out_d[:, g * GRP:(g + 1) * GRP, :],
                        in_=stage[:].rearrange("p (g n) -> p g n", g=GRP))

    nc.compile()
    return nc


def _get_program():
    global _PROG
    if _PROG is None:
        _PROG = _build_program()
    return _PROG


def _host_prep(memory, od_mat, walks, W_rw, b_rw, W1, b1, W2, b2,
               gru_Wi, gru_bi, gru_Wh, gru_bh, Wp1, bp1, Wp2, bp2):
    import ml_dtypes
    f = np.float32
    bf = ml_dtypes.bfloat16
    e4 = ml_dtypes.float8_e4m3fn
    memory = np.ascontiguousarray(np.asarray(memory), dtype=f)
    od_mat = np.asarray(od_mat)
    walks = np.asarray(walks).astype(np.int32)
    W_rw = np.asarray(W_rw, dtype=f); b_rw = np.asarray(b_rw, dtype=f)
    W1 = np.asarray(W1, dtype=f); b1 = np.asarray(b1, dtype=f)
    W2 = np.asarray(W2, dtype=f); b2 = np.asarray(b2, dtype=f)
    gru_Wi = np.asarray(gru_Wi, dtype=f); gru_bi = np.asarray(gru_bi, dtype=f)
    gru_Wh = np.asarray(gru_Wh, dtype=f); gru_bh = np.asarray(gru_bh, dtype=f)
    Wp1 = np.asarray(Wp1, dtype=f); bp1 = np.asarray(bp1, dtype=f)
    Wp2 = np.asarray(Wp2, dtype=f); bp2 = np.asarray(bp2, dtype=f)

    # layer-1 weights, column-permuted to [od | dest | walk] with W_rw and the
    # 1/8 mean folded into the walk block; HID padded to 2176; whole block
    # scaled x16 so the fp8 od weights sit in e4m3 normal range (1/16 folded
    # into W2; exact since relu(16x)=16relu(x))
    W1od = W1[:, MD:MD + N]
    W1dest = W1[:, 0:MD]
    W1rw = W1[:, MD + N:]
    W1g = (W1rw @ W_rw) / np.float32(8.0)
    W1p = np.concatenate([W1od, W1dest, W1g], axis=1) * np.float32(WSCALE)
    W1pT = np.zeros((33 * 128, HIDP), dtype=f)
    W1pT[:, :HID] = W1p.T
    # w1h[h][p, k*128+c] = W1pT[k*128+p, h*128+c] for the 32 od k-tiles
    # (pairs of adjacent k-tiles feed one DoubleRow matmul);
    # the mixed k-tile (rows 4096:4224) is its own resident tensor w1m
    w1h = np.ascontiguousarray(
        W1pT[:32 * 128].reshape(32, 128, HT, 128)
        .transpose(2, 1, 0, 3).reshape(HT, 128, 32 * 128).astype(e4))
    w1m = np.ascontiguousarray(W1pT[32 * 128:].astype(bf))  # [128, 2176]

    b1p = np.zeros(HIDP, dtype=f)
    b1p[:HID] = (b1 + W1rw @ b_rw) * np.float32(WSCALE)

    W2tp = np.zeros((HIDP, MSG), dtype=f)
    W2tp[:HID] = W2.T / np.float32(WSCALE)
    # w2t[p, h*64+c] = W2tp[h*128+p, c]
    w2t = np.ascontiguousarray(
        W2tp.reshape(HT, 128, MSG).transpose(1, 0, 2).reshape(128, HT * MSG)
        .astype(bf))

    def pad128(v):
        o = np.zeros(128, dtype=f)
        o[:v.shape[0]] = v
        return o

    # b2 folded through the GRU input weights: gi = Wi@(msg'+b2)+bi
    gbi_f = gru_bi + gru_Wi @ b2

    # biases packed as [128 partitions, 64 columns]
    biases = np.zeros((64, 128), dtype=f)
    biases[0:HT] = b1p.reshape(HT, 128)
    grz = gbi_f[:128] + gru_bh[:128]
    biases[17] = pad128(grz[:64])      # r gate bias
    biases[18] = pad128(grz[64:])      # z gate bias
    biases[19] = pad128(gbi_f[128:])
    biases[20] = pad128(gru_bh[128:])
    biases[21] = np.concatenate([bp1, bp1])  # duplicated for act2 row-tiling
    biases[22:54] = bp2.reshape(32, 128)
    biases = np.ascontiguousarray(biases.T)                    # [128, 64]

    WiT = np.ascontiguousarray(gru_Wi.T)                       # [64, 192]
    shared = {
        "mem": memory,
        "w1h": w1h,
        "w1m": w1m,
        "w2t": w2t,
        # Wi rows duplicated: gi = [Wi;Wi] @ [msgA;msgB] (K=128)
        "wi2": np.ascontiguousarray(
            np.concatenate([WiT, WiT], axis=0).astype(bf)),    # [128, 192]
        "wh": np.ascontiguousarray(gru_Wh.T.astype(bf)),       # [64, 192]
        # [Wp1|Wp1]: act lands duplicated on partitions 0:64/64:128
        "wp1x": np.ascontiguousarray(
            np.concatenate([Wp1.T, Wp1.T], axis=1).astype(bf)),  # [64, 128]
        # Wp2T duplicated on partitions 0:64/64:128 for row-tiled pairs
        "wp2d": np.ascontiguousarray(
            np.concatenate([Wp2.T, Wp2.T], axis=0).astype(bf)),  # [128, 4096]
        "biases": biases,
        "ident": np.eye(128, dtype=f),
    }
    in_maps = []
    for c in range(NC):
        sl = slice(c * R, (c + 1) * R)
        odc_np = np.asarray(od_mat[sl], dtype=f)
        # odv[p, k*R+n] = od[c*R+n, k*128+p]
        odv = np.ascontiguousarray(
            odc_np.T.reshape(32, 128, R).transpose(1, 0, 2).reshape(128, 32 * R)
            .astype(e4))
        if USE_DMA_GATHER:
            # dma_gather idx layout: idx[(t*WL+j)*128+p] = walks[t*128+p, j],
            # wrap-16, replicated across the 8 Q7 core stripes
            wkc = walks[sl].reshape(NT, 128, WL).transpose(0, 2, 1).reshape(-1)
            widx = np.ascontiguousarray(np.tile(
                wkc.reshape(-1, 16).T.astype(np.int16), (8, 1)))  # [128, 256]
        else:
            # widx[p, t*WL+j] = walks[c*R + t*128 + p, j]
            widx = np.ascontiguousarray(
                walks[sl].reshape(NT, 128, WL).transpose(1, 0, 2)
                .reshape(128, NT * WL))
        memT = np.ascontiguousarray(memory[sl].T)
        in_maps.append(dict(
            shared,
            memT=np.ascontiguousarray(memT.astype(bf)),
            odv=odv,
            widx=widx,
        ))
    return in_maps


def _assemble(results):
    od = np.empty((N, N), dtype=np.float32)
    for c in range(NC):
        # outm[p, m, n] = od[c*R+n, m*128+p]
        od[c * R:(c + 1) * R, :] = (
            results[c]["outm"].astype(np.float32).transpose(2, 1, 0).reshape(R, N))
    return od


def _install_ntff_shim():
    """The agent image's antenv lacks axon_hooks, so trace=True dies on
    import. Recreate the module with the ctypes-based NTFF hook that
    trn_agent_boot would have registered."""
    import sys
    import types
    if "antenv.axon_hooks" in sys.modules:
        return
    from trn_agent_boot.trn_boot import _ntff_profile_via_ctypes
    hook = _ntff_profile_via_ctypes("/opt/axon/libaxon_pjrt.so")
    mod = types.ModuleType("antenv.axon_hooks")
    mod._hook = hook
    mod.get_axon_ntff_profile_hook = lambda: mod._hook
    mod.set_axon_ntff_profile_hook = lambda h: setattr(mod, "_hook", h)
    sys.modules["antenv.axon_hooks"] = mod


def run(inputs, trace=False):
    """Run on 8 NeuronCores; returns (od [N,N] f32, BassKernelResults)."""
    from concourse.bass_utils import run_bass_kernel_spmd
    if trace:
        try:
            _install_ntff_shim()
        except Exception as e:
            print(f"ntff shim failed ({e}); running without trace")
            trace = False
    nc = _get_program()
    in_maps = _host_prep(**inputs)
    res = run_bass_kernel_spmd(nc, in_maps, list(range(NC)), trace=trace)
    return _assemble(res.results), res


def kernel(**inputs):
    od, _ = run(inputs)
    return od


# revision 66
# speedup vs baseline: 1.1550x; 1.1550x over previous
"""Trainium2 Bass kernel for nn_DiscreteModel (GNN message passing).

Strategy: shard by node rows across 8 cores (512 rows each). All per-node
tensors are kept feature-major ([feature, node]) on-chip so the contraction
dim of every matmul sits on SBUF partitions. The host pre-transposes the
od_mat shard and all weights, folds the random-walk projection W_rw and the
1/8 mean into the layer-1 weight block, and pads HID 2112 -> 2176.

v3 (~125us, from the 158us v2). The PE issues a warm fp8-DoubleRow matmul
every ~216ns (2.4GHz; the 380ns trace "duration" is issue-to-drain), so the
od x W1 block floors at ~59us and everything else must hide behind it:
  head   : od loaded as 8 chunk tiles (2KB partition lines; per-chunk
           matmul gating, odc1 queued ahead of w1tB0 -> first DR matmul
           ~8.4us); w1 h-tiles split in halves across the sync/scalar
           queues; the w1m/w2t constant loads are split across slots
           5/6/7 so their backlog stays under the per-slot DMA slack;
           4 zero-DR warmup matmuls fill the PE until od chunk 0 lands.
  gather : 32 indirect DMAs (~1.1us of gpsimd SWDGE ucode each, ends
           ~56us; dma_gather would pay a ~13us Q7 ucode lib load first).
           gpsimd carries NOTHING else until the output DMAs.
  spill  : h<9 evacuate their od partial to SBUF (scalar ACT) and replay
           after the mixed k-tile exists (lag-2 finalize for h>=9, one
           replay per slot from slot 11, short drain; transposes emitted
           at slot 10, after the walk sums land ~60us). The walk-sum
           transposes read identx = ident + 0*odp[8] -- a data-dep pin so
           the Tile scheduler (sim-driven, reorders freely) cannot hoist
           them and their semaphore stalls into the early od stream.
  L2     : col-tiled pairs (even h -> psum[0:64], odd h -> [64:128],
           concurrent); halves combined for free via duplicated GRU Wi
           rows (gi = [Wi;Wi] @ [msgA;msgB], K=128 costs the same).
  relu   : alternates scalar ACT / vector add+max so neither engine paces
           the finalize chain; replay adds on vector, replay relu scalar.
  tail   : GRU memory operand in bf16 from the resident memT tile; wp1
           output M=128 with [Wp1|Wp1] so act lands duplicated on
           partitions 0:64/64:128; pred runs as 16 row-tiled PAIRS
           (tile_position (0,0)/(64,0), concurrent); evacuations alternate
           scalar/vector into bf16 staging; output DMA per 4 m-tiles on
           sync/gpsimd with [128, 32, R] DRAM layout (4KB lines).
fp8    : the od x W1 block (K=4096 of 4224) runs in fp8e4 DoubleRow mode.
         W1od is scaled x16 on host (relu(16x)=16relu(x); 1/16 folded
         into W2). Rel err ~1.24e-2 vs the 2e-2 gate.
Note: the axon TRN2 fleet drifts run-to-run (same NEFF 124..151us);
compare kernels only back-to-back within one window, min-of-3.
"""

import numpy as np

import concourse.bass as bass
import concourse.bacc as bacc
import concourse.tile as tile
from concourse import mybir

N = 4096        # nodes
MD = 64         # memory dim
MSG = 64        # message dim
WL = 8          # walk length
HID = 2112
HT = 17         # h-tiles (HID padded to 17*128 = 2176)
HIDP = HT * 128
NC = 8          # cores
R = N // NC     # rows (nodes) per core = 512
NT = R // 128   # node tiles per core = 4
F32 = mybir.dt.float32
F32R = mybir.dt.float32r
BF16 = mybir.dt.bfloat16
FP8 = mybir.dt.float8e4
I16 = mybir.dt.int16
I32 = mybir.dt.int32
WSCALE = 16.0   # W1 block scale so fp8 weights sit in e4m3 normal range
USE_DMA_GATHER = False   # Q7 SWDGE gather pays ~13us ucode lib load; the 32
                         # indirect DMAs (1.1us gpsimd ucode each) end sooner
N_WARM = 4      # zero DR matmuls fill the PE only until the first od
                # chunk lands (~8.7us); more would delay the real stream
LAG = 2         # h-tiles between od part and mixed finalize

_PROG = None


def _build_program():
    nc = bacc.Bacc("TRN2", target_bir_lowering=False, debug=False, num_devices=NC)

    # ---- DRAM I/O (all pre-laid-out on host, partition-major) ----
    mem_d = nc.dram_tensor("mem", [N, MD], F32, kind="ExternalInput").ap()
    memT_d = nc.dram_tensor("memT", [MD, R], BF16, kind="ExternalInput").ap()
    od_d = nc.dram_tensor("odv", [128, 32 * R], FP8, kind="ExternalInput").ap()
    if USE_DMA_GATHER:
        widx_d = nc.dram_tensor("widx", [128, NT * WL * 128 // 16], I16,
                                kind="ExternalInput").ap()
    else:
        widx32_d = nc.dram_tensor("widx", [128, NT * WL], I32,
                                  kind="ExternalInput").ap()
    w1h_d = nc.dram_tensor("w1h", [HT, 128, 32 * 128], FP8, kind="ExternalInput").ap()
    w1m_d = nc.dram_tensor("w1m", [128, HT * 128], BF16, kind="ExternalInput").ap()
    w2t_d = nc.dram_tensor("w2t", [128, HT * MSG], BF16, kind="ExternalInput").ap()
    wi2_d = nc.dram_tensor("wi2", [128, 3 * MD], BF16, kind="ExternalInput").ap()
    wh_d = nc.dram_tensor("wh", [MD, 3 * MD], BF16, kind="ExternalInput").ap()
    wp1_d = nc.dram_tensor("wp1x", [MD, 128], BF16, kind="ExternalInput").ap()
    wp2_d = nc.dram_tensor("wp2d", [128, N], BF16, kind="ExternalInput").ap()
    bias_d = nc.dram_tensor("biases", [128, 64], F32, kind="ExternalInput").ap()
    ident_d = nc.dram_tensor("ident", [128, 128], F32, kind="ExternalInput").ap()
    out_d = nc.dram_tensor("outm", [128, 32, R], BF16, kind="ExternalOutput").ap()

    AF = mybir.ActivationFunctionType
    DR = mybir.MatmulPerfMode.DoubleRow
    HK = 8 * 256            # half of a w1 h-tile (k-pairs 0..7)

    with tile.TileContext(nc) as tc:
        with (
            tc.tile_pool(name="consts", bufs=1) as consts,
            tc.tile_pool(name="w1p", bufs=3) as w1p,
            tc.tile_pool(name="gp", bufs=2) as gp,
            tc.tile_pool(name="hp", bufs=4) as hp,
            tc.tile_pool(name="gates", bufs=1) as gates,
            tc.tile_pool(name="ostg", bufs=3) as ostg,
            tc.tile_pool(name="pmm", bufs=5, space="PSUM") as pmm,
            tc.tile_pool(name="pacc", bufs=1, space="PSUM") as pacc,
            tc.tile_pool(name="ptr", bufs=1, space="PSUM") as ptr,
        ):
            # ---- walk indices first: gather feeds the mixed k-tile.
            # Split per node-tile so the first indirect DMA starts as soon
            # as its own 4KB of indices lands.
            if USE_DMA_GATHER:
                wk = consts.tile([128, NT * WL * 128 // 16], I16, tag="wk")
                nc.gpsimd.dma_start(out=wk[:], in_=widx_d[:])
            else:
                wk = consts.tile([128, NT * WL], I32, tag="wk")
                for t in range(NT):
                    nc.gpsimd.dma_start(out=wk[:, t * WL:(t + 1) * WL],
                                        in_=widx32_d[:, t * WL:(t + 1) * WL])

            # head DMA: the first DR matmul needs w1 h0 front half + od chunk
            # 0 only -> both lead their queues; od goes in 8 chunks of 2
            # k-pairs (2KB partition lines for full DMA efficiency), even on
            # sync, odd on scalar, so matmul k gates on chunk k//2.
            w1tA = [None] * HT
            w1tB = [None] * HT
            w1tA[0] = w1p.tile([128, HK], FP8, tag="w1tA", name="w1tA0")
            nc.sync.dma_start(out=w1tA[0][:], in_=w1h_d[0][:, :HK])
            odc = []
            for c in range(8):
                t = consts.tile([128, 4 * R], FP8, tag=f"odc{c}",
                                name=f"odc{c}")
                odc.append(t)
            nc.scalar.dma_start(out=odc[0][:], in_=od_d[:, 0:4 * R])
            nc.scalar.dma_start(out=odc[1][:], in_=od_d[:, 4 * R:8 * R])
            w1tB[0] = w1p.tile([128, HK], FP8, tag="w1tB", name="w1tB0")
            nc.scalar.dma_start(out=w1tB[0][:], in_=w1h_d[0][:, HK:])
            for c in range(2, 8):
                eng = nc.sync if c % 2 == 0 else nc.scalar
                eng.dma_start(out=odc[c][:],
                              in_=od_d[:, c * 4 * R:(c + 1) * 4 * R])
            for h in (1, 2):
                w1tA[h] = w1p.tile([128, HK], FP8, tag="w1tA", name=f"w1tA{h}")
                nc.sync.dma_start(out=w1tA[h][:], in_=w1h_d[h][:, :HK])
                w1tB[h] = w1p.tile([128, HK], FP8, tag="w1tB", name=f"w1tB{h}")
                nc.scalar.dma_start(out=w1tB[h][:], in_=w1h_d[h][:, HK:])

            # Q7 SWDGE gather for all 4096 walk rows:
            # gare[p, (t*WL+j)*MD : +MD] = mem[walks[t*128+p, j]]
            gare = consts.tile([128, NT * WL * MD], F32, tag="gare")
            if USE_DMA_GATHER:
                # >=2048 idxs per instruction hangs the Q7 ucode on HW;
                # 4x1024 (one per node-tile) costs ~1.2us SWDGE each
                for t in range(NT):
                    nc.gpsimd.dma_gather(
                        gare[:, t * WL * MD:(t + 1) * WL * MD].rearrange(
                            "p (g d) -> p g d", g=WL),
                        mem_d[:], wk[:, t * WL * 8:(t + 1) * WL * 8],
                        WL * 128, WL * 128, MD,
                    )
            else:
                for t in range(NT):
                    for j in range(WL):
                        o = (t * WL + j) * MD
                        nc.gpsimd.indirect_dma_start(
                            out=gare[:, o:o + MD],
                            out_offset=None,
                            in_=mem_d[:],
                            in_offset=bass.IndirectOffsetOnAxis(
                                ap=wk[:, t * WL + j:t * WL + j + 1], axis=0),
                        )

            # PE warmup: zero DR matmuls from ~6.5us pull HAM to full clock
            # before the real stream starts (idle >3.4us re-throttles)
            zx = consts.tile([128, 2 * R], FP8, tag="zx")
            nc.vector.memset(zx[:], 0)
            pdum = pmm.tile([128, R], F32, tag="mm")
            for _ in range(N_WARM):
                nc.tensor.matmul(
                    out=pdum[:],
                    lhsT=zx[:, 0:256].rearrange("p (two m) -> p two m", two=2),
                    rhs=zx[:].rearrange("p (two n) -> p two n", two=2),
                    start=True, stop=True, perf_mode=DR,
                )

            # identity from DRAM: gpsimd must stay free for the gather ucode
            ident = consts.tile([128, 128], F32, tag="ident")
            nc.scalar.dma_start(out=ident[:], in_=ident_d[:])
            biasp = consts.tile([128, 64], F32, tag="biasp")
            nc.scalar.dma_start(out=biasp[:], in_=bias_d[:])

            # mixed rawT k-tile: [0:64] = memT shard, [64:128] = GsT (walk sums)
            mixed = consts.tile([128, R], BF16, tag="mixed")
            nc.scalar.dma_start(out=mixed[0:MD, :], in_=memT_d[:])

            # constant tiles; their DMAs are emitted inside the h-loop so
            # they queue behind the od/w1 head flood (needed ~55us onward)
            w1m_sb = consts.tile([128, HT * 128], BF16, tag="w1m")
            w2t_sb = consts.tile([128, HT * MSG], BF16, tag="w2t")
            wh_sb = consts.tile([MD, 3 * MD], BF16, tag="wh")
            wi2_sb = consts.tile([128, 3 * MD], BF16, tag="wi2")
            wp1_sb = consts.tile([MD, 128], BF16, tag="wp1x")
            wp2_sb = consts.tile([128, N], BF16, tag="wp2d")

            # preload the sigmoid/tanh ACT table while the head is DMA-paced
            # (otherwise a 1.28us ACT_TABLE_LOAD lands on the GRU chain)
            warm = gates.tile([MD, 4], F32, tag="warm")
            nc.scalar.activation(warm[:, 0:2], biasp[0:MD, 0:2], AF.Sigmoid)
            nc.scalar.activation(warm[:, 2:4], biasp[0:MD, 0:2], AF.Tanh)

            # walk sums on DVE (gated on the gather), one per node-tile
            m1s = [None] * NT
            for t in range(NT):
                ga3 = gare[:, t * WL * MD:(t + 1) * WL * MD].rearrange(
                    "p (j d) -> p j d", j=WL)
                m4 = gp.tile([128, 4 * MD], F32, tag="m4")
                m43 = m4[:].rearrange("p (j d) -> p j d", j=4)
                nc.vector.tensor_add(out=m43, in0=ga3[:, 0:4, :], in1=ga3[:, 4:8, :])
                m2 = gp.tile([128, 2 * MD], F32, tag="m2")
                m23 = m2[:].rearrange("p (j d) -> p j d", j=2)
                nc.vector.tensor_add(out=m23, in0=m43[:, 0:2, :], in1=m43[:, 2:4, :])
                m1t = gp.tile([128, MD], F32, tag=f"m1_{t}")
                nc.vector.tensor_add(out=m1t[:], in0=m2[:, 0:MD],
                                     in1=m2[:, MD:2 * MD])
                m1s[t] = m1t

            mixed_r = mixed[:]

            # ---- layer 1 (fp8 DoubleRow); the gather (4x ~8.6us Q7 ucode)
            # only completes ~44us in, so h < SPILL spill their od partial to
            # SBUF (freeing the PSUM bank) and replay one per slot once the
            # mixed tile exists; h >= SPILL run a lag-2 finalize.
            SPILL = 9
            psL2 = pacc.tile([128, R], F32, tag="l2")
            odp = consts.tile([128, SPILL * R], F32, tag="odp")
            identx = consts.tile([128, 128], F32, tag="identx")
            pss = {}
            hids = {}
            l2n = [0, 0]
            L2N = [9, 8]   # even/odd L2 stream lengths

            def emit_l2(h):
                half = h % 2
                nc.tensor.matmul(
                    out=psL2[half * 64:(half + 1) * 64, :],
                    lhsT=w2t_sb[:, h * MSG:(h + 1) * MSG],
                    rhs=hids.pop(h)[:],
                    start=(l2n[half] == 0), stop=(l2n[half] == L2N[half] - 1),
                )
                l2n[half] += 1

            def emit_relu(h, src):
                # alternate relu between scalar ACT and vector (add-bias,
                # max 0) so neither engine paces the finalize/replay chain
                hid = hp.tile([128, R], BF16, tag="hid")
                if h % 2 == 0:
                    nc.scalar.activation(hid[:], src, AF.Relu,
                                         bias=biasp[:, h:h + 1])
                else:
                    nc.vector.tensor_scalar(
                        out=hid[:], in0=src, scalar1=biasp[:, h:h + 1],
                        scalar2=0.0, op0=mybir.AluOpType.add,
                        op1=mybir.AluOpType.max)
                hids[h] = hid

            def finalize(h):
                ps = pss.pop(h)
                nc.tensor.matmul(
                    out=ps[:], lhsT=w1m_sb[:, h * 128:(h + 1) * 128],
                    rhs=mixed_r, start=False, stop=True,
                )
                emit_relu(h, ps[:])
                emit_l2(h)

            def replay(h):
                ps = pmm.tile([128, R], F32, tag="mm")
                nc.tensor.matmul(
                    out=ps[:], lhsT=w1m_sb[:, h * 128:(h + 1) * 128],
                    rhs=mixed_r, start=True, stop=True,
                )
                pre = gp.tile([128, R], F32, tag="clt")
                nc.vector.tensor_add(out=pre[:], in0=ps[:],
                                     in1=odp[:, h * R:(h + 1) * R])
                # vector already carries the add: replay relu goes to scalar
                hid = hp.tile([128, R], BF16, tag="hid")
                nc.scalar.activation(hid[:], pre[:], AF.Relu,
                                     bias=biasp[:, h:h + 1])
                hids[h] = hid
                emit_l2(h)

            for h in range(HT):
                if h >= 3:
                    w1tA[h] = w1p.tile([128, HK], FP8, tag="w1tA",
                                       name=f"w1tA{h}")
                    w1tB[h] = w1p.tile([128, HK], FP8, tag="w1tB",
                                       name=f"w1tB{h}")
                    engA = nc.sync if h % 2 == 1 else nc.scalar
                    engB = nc.scalar if h % 2 == 1 else nc.sync
                    engA.dma_start(out=w1tA[h][:], in_=w1h_d[h][:, :HK])
                    engB.dma_start(out=w1tB[h][:], in_=w1h_d[h][:, HK:])
                ps = pmm.tile([128, R], F32, tag="mm")
                for k in range(16):
                    wt = w1tA[h] if k < 8 else w1tB[h]
                    nc.tensor.matmul(
                        out=ps[:],
                        lhsT=wt[:, (k % 8) * 256:(k % 8 + 1) * 256].rearrange(
                            "p (two m) -> p two m", two=2),
                        rhs=odc[k // 2][:, (k % 2) * 2 * R:(k % 2 + 1) * 2 * R]
                        .rearrange("p (two n) -> p two n", two=2),
                        start=(k == 0), stop=(h < SPILL),
                        perf_mode=DR,
                    )
                if h < SPILL:
                    # evacuate pre-activation od partial (scalar; the vector
                    # stream is held by the gather-gated walk sums)
                    nc.scalar.activation(odp[:, h * R:(h + 1) * R], ps[:],
                                         AF.Identity)
                else:
                    pss[h] = ps
                if h == SPILL - 1:
                    # identx = ident + 0*odp[h]: a scheduler pin — the mix
                    # transposes read identx, so no schedule can hoist them
                    # (and their semaphore stalls) into the early od stream
                    tmpid = gp.tile([128, 128], F32, tag="tmpid")
                    nc.vector.tensor_scalar_mul(
                        out=tmpid[:], in0=odp[:, h * R:h * R + 128],
                        scalar1=0.0)
                    nc.vector.tensor_add(out=identx[:], in0=ident[:],
                                         in1=tmpid[:])
                if h == 5:
                    # replays (h<9) read the low half first
                    nc.scalar.dma_start(out=w1m_sb[:, :9 * 128],
                                        in_=w1m_d[:, :9 * 128])
                if h == 6:
                    nc.sync.dma_start(out=w2t_sb[:], in_=w2t_d[:])
                if h == 7:
                    nc.scalar.dma_start(out=w1m_sb[:, 9 * 128:],
                                        in_=w1m_d[:, 9 * 128:])
                if h == 12:
                    # tail-only constants: late so they never delay the w1
                    # stream that gates the od matmuls
                    nc.scalar.dma_start(out=wh_sb[:], in_=wh_d[:])
                    nc.scalar.dma_start(out=wi2_sb[:], in_=wi2_d[:])
                    nc.scalar.dma_start(out=wp1_sb[:], in_=wp1_d[:])
                    nc.sync.dma_start(out=wp2_sb[:], in_=wp2_d[:])
                if h == 14:
                    # h_n = memT @ Wh_n + bias depends only on memT; do it in
                    # the ramp where the PE has slack
                    ps_hn = pmm.tile([MD, R], F32, tag="mm")
                    nc.tensor.matmul(out=ps_hn[:], lhsT=wh_sb[:, 128:192],
                                     rhs=mixed[0:MD, :], start=True, stop=True)
                    hnb = gates.tile([MD, R], F32, tag="hnb")
                    nc.vector.tensor_scalar_add(out=hnb[:], in0=ps_hn[:],
                                                scalar1=biasp[0:MD, 20:21])
                if h == SPILL + 1:
                    # GsT transposes; PE reaches them ~62us, sums done ~60us
                    for t in range(NT):
                        tr = ptr.tile([MD, 128], F32, tag="tr")
                        nc.tensor.transpose(out=tr[:], in_=m1s[t][:],
                                            identity=identx[:])
                        nc.vector.tensor_copy(
                            out=mixed[MD:128, t * 128:(t + 1) * 128], in_=tr[:])
                if h >= SPILL + LAG:
                    finalize(h - LAG)
                    replay(h - SPILL - LAG)
            finalize(HT - 2)
            finalize(HT - 1)
            for h in range(min(HT - SPILL - LAG, SPILL), SPILL):
                replay(h)

            # msg (pre-b2, which is folded into the GRU input bias):
            # both L2 col halves evacuated in one op; the GRU adds them by
            # using duplicated Wi rows (K=128 costs the same as K=64)
            msg2x = gates.tile([128, R], BF16, tag="msg2x")
            nc.scalar.activation(msg2x[:], psL2[:], AF.Identity)
            msg_r = msg2x[:]
            memT_r = mixed[0:MD, :]

            # ---- GRU + prediction, column-split so the serial ACT/DVE chain
            #      pipelines across halves and the PE never idles >3.4us.
            ps_r = pmm.tile([MD, R], F32, tag="mm")
            nc.tensor.matmul(out=ps_r[:], lhsT=wi2_sb[:, 0:MD], rhs=msg_r,
                             start=True, stop=False)
            nc.tensor.matmul(out=ps_r[:], lhsT=wh_sb[:, 0:MD], rhs=memT_r,
                             start=False, stop=True)
            ps_z = pmm.tile([MD, R], F32, tag="mm")
            nc.tensor.matmul(out=ps_z[:], lhsT=wi2_sb[:, MD:128], rhs=msg_r,
                             start=True, stop=False)
            nc.tensor.matmul(out=ps_z[:], lhsT=wh_sb[:, MD:128], rhs=memT_r,
                             start=False, stop=True)
            ps_in = pmm.tile([MD, R], F32, tag="mm")
            nc.tensor.matmul(out=ps_in[:], lhsT=wi2_sb[:, 128:192], rhs=msg_r,
                             start=True, stop=True)
            r_t = gates.tile([MD, R], F32, tag="r_t")
            z_t = gates.tile([MD, R], F32, tag="z_t")
            rhn = gates.tile([MD, R], F32, tag="rhn")
            npre = gates.tile([MD, R], F32, tag="npre")
            n_t = gates.tile([MD, R], F32, tag="n_t")
            zc_t = gates.tile([MD, R], F32, tag="zc_t")
            zm_t = gates.tile([MD, R], BF16, tag="zm_t")
            ncz = gates.tile([MD, R], BF16, tag="ncz")
            ps_pred = pacc.tile([128, R], F32, tag="pred")
            act2 = gates.tile([128, R], BF16, tag="act2")
            HR = R // 2
            for x in range(2):
                cs = slice(x * HR, (x + 1) * HR)
                nc.scalar.activation(r_t[:, cs], ps_r[:, cs], AF.Sigmoid,
                                     bias=biasp[0:MD, 17:18])
                nc.scalar.activation(z_t[:, cs], ps_z[:, cs], AF.Sigmoid,
                                     bias=biasp[0:MD, 18:19])
                # upd = (1-z)*n + z*mem = zc*n + zm; zc/zm go on vector right
                # after the z sigmoid (gpsimd would pay a Q7 ucode lib swap
                # after the gathers that serializes the whole chain)
                nc.vector.tensor_scalar(out=zc_t[:, cs], in0=z_t[:, cs],
                                        scalar1=-1.0, scalar2=1.0,
                                        op0=mybir.AluOpType.mult,
                                        op1=mybir.AluOpType.add)
                nc.vector.tensor_mul(out=zm_t[:, cs], in0=z_t[:, cs],
                                     in1=memT_r[:, cs])
                # upd = ncz + zm is absorbed into wp1 by linearity:
                # Wp1@(ncz+zm) = Wp1@ncz + Wp1@zm. The zm part runs right
                # after the z sigmoid, off the r->tanh critical chain
                nc.tensor.matmul(out=ps_pred[:, cs], lhsT=wp1_sb[:],
                                 rhs=zm_t[:, cs], start=True, stop=False)
                nc.vector.tensor_mul(out=rhn[:, cs], in0=r_t[:, cs], in1=hnb[:, cs])
                nc.vector.tensor_add(out=npre[:, cs], in0=ps_in[:, cs], in1=rhn[:, cs])
                nc.scalar.activation(n_t[:, cs], npre[:, cs], AF.Tanh,
                                     bias=biasp[0:MD, 19:20])
                nc.vector.tensor_mul(out=ncz[:, cs], in0=zc_t[:, cs], in1=n_t[:, cs])
                # [Wp1|Wp1] -> act duplicated on partitions 0:64 / 64:128 so
                # pred pairs can row-tile
                nc.tensor.matmul(out=ps_pred[:, cs], lhsT=wp1_sb[:], rhs=ncz[:, cs],
                                 start=False, stop=True)
                nc.scalar.activation(act2[:, cs], ps_pred[:, cs], AF.Relu,
                                     bias=biasp[:, 21:22])

            # ---- prediction m-loop: 16 row-tiled PAIRS (tile_position
            #      (0,0)/(64,0), concurrent on the PE); evacuations alternate
            #      scalar/vector; output staged bf16, 0.5MB DMAs
            GRP = 4
            for m2 in range(16):
                m0, m1 = 2 * m2, 2 * m2 + 1
                psA = pmm.tile([128, R], F32, tag="mm")
                psB = pmm.tile([128, R], F32, tag="mm")
                nc.tensor.matmul(out=psA[:],
                                 lhsT=wp2_sb[0:64, m0 * 128:(m0 + 1) * 128],
                                 rhs=act2[0:64, :], start=True, stop=True)
                nc.tensor.matmul(out=psB[:],
                                 lhsT=wp2_sb[64:128, m1 * 128:(m1 + 1) * 128],
                                 rhs=act2[64:128, :], start=True, stop=True)
                if m0 % GRP == 0:
                    stage = ostg.tile([128, GRP * R], BF16, tag="stage")
                slA = stage[:, (m0 % GRP) * R:(m0 % GRP + 1) * R]
                slB = stage[:, (m1 % GRP) * R:(m1 % GRP + 1) * R]
                nc.scalar.activation(slA, psA[:], AF.Identity,
                                     bias=biasp[:, 22 + m0:23 + m0])
                nc.vector.tensor_scalar_add(out=slB, in0=psB[:],
                                            scalar1=biasp[:, 22 + m1:23 + m1])
                if m1 % GRP == GRP - 1:
                    g = m1 // GRP
                    oeng = nc.sync if g % 2 == 0 else nc.gpsimd
                    oeng.dma_start(
                        out=out_d[:, g * GRP:(g + 1) * GRP, :],
                        in_=stage[:].rearrange("p (g n) -> p g n", g=GRP))

    nc.compile()
    return nc


def _get_program():
    global _PROG
    if _PROG is None:
        _PROG = _build_program()
    return _PROG


def _host_prep(memory, od_mat, walks, W_rw, b_rw, W1, b1, W2, b2,
               gru_Wi, gru_bi, gru_Wh, gru_bh, Wp1, bp1, Wp2, bp2):
    import ml_dtypes
    f = np.float32
    bf = ml_dtypes.bfloat16
    e4 = ml_dtypes.float8_e4m3fn
    memory = np.ascontiguousarray(np.asarray(memory), dtype=f)
    od_mat = np.asarray(od_mat)
    walks = np.asarray(walks).astype(np.int32)
    W_rw = np.asarray(W_rw, dtype=f); b_rw = np.asarray(b_rw, dtype=f)
    W1 = np.asarray(W1, dtype=f); b1 = np.asarray(b1, dtype=f)
    W2 = np.asarray(W2, dtype=f); b2 = np.asarray(b2, dtype=f)
    gru_Wi = np.asarray(gru_Wi, dtype=f); gru_bi = np.asarray(gru_bi, dtype=f)
    gru_Wh = np.asarray(gru_Wh, dtype=f); gru_bh = np.asarray(gru_bh, dtype=f)
    Wp1 = np.asarray(Wp1, dtype=f); bp1 = np.asarray(bp1, dtype=f)
    Wp2 = np.asarray(Wp2, dtype=f); bp2 = np.asarray(bp2, dtype=f)

    # layer-1 weights, column-permuted to [od | dest | walk] with W_rw and the
    # 1/8 mean folded into the walk block; HID padded to 2176; whole block
    # scaled x16 so the fp8 od weights sit in e4m3 normal range (1/16 folded
    # into W2; exact since relu(16x)=16relu(x))
    W1od = W1[:, MD:MD + N]
    W1dest = W1[:, 0:MD]
    W1rw = W1[:, MD + N:]
    W1g = (W1rw @ W_rw) / np.float32(8.0)
    W1p = np.concatenate([W1od, W1dest, W1g], axis=1) * np.float32(WSCALE)
    W1pT = np.zeros((33 * 128, HIDP), dtype=f)
    W1pT[:, :HID] = W1p.T
    # w1h[h][p, k*128+c] = W1pT[k*128+p, h*128+c] for the 32 od k-tiles
    # (pairs of adjacent k-tiles feed one DoubleRow matmul);
    # the mixed k-tile (rows 4096:4224) is its own resident tensor w1m
    w1h = np.ascontiguousarray(
        W1pT[:32 * 128].reshape(32, 128, HT, 128)
        .transpose(2, 1, 0, 3).reshape(HT, 128, 32 * 128).astype(e4))
    w1m = np.ascontiguousarray(W1pT[32 * 128:].astype(bf))  # [128, 2176]

    b1p = np.zeros(HIDP, dtype=f)
    b1p[:HID] = (b1 + W1rw @ b_rw) * np.float32(WSCALE)

    W2tp = np.zeros((HIDP, MSG), dtype=f)
    W2tp[:HID] = W2.T / np.float32(WSCALE)
    # w2t[p, h*64+c] = W2tp[h*128+p, c]
    w2t = np.ascontiguousarray(
        W2tp.reshape(HT, 128, MSG).transpose(1, 0, 2).reshape(128, HT * MSG)
        .astype(bf))

    def pad128(v):
        o = np.zeros(128, dtype=f)
        o[:v.shape[0]] = v
        return o

    # b2 folded through the GRU input weights: gi = Wi@(msg'+b2)+bi
    gbi_f = gru_bi + gru_Wi @ b2

    # biases packed as [128 partitions, 64 columns]
    biases = np.zeros((64, 128), dtype=f)
    biases[0:HT] = b1p.reshape(HT, 128)
    grz = gbi_f[:128] + gru_bh[:128]
    biases[17] = pad128(grz[:64])      # r gate bias
    biases[18] = pad128(grz[64:])      # z gate bias
    biases[19] = pad128(gbi_f[128:])
    biases[20] = pad128(gru_bh[128:])
    biases[21] = np.concatenate([bp1, bp1])  # duplicated for act2 row-tiling
    biases[22:54] = bp2.reshape(32, 128)
    biases = np.ascontiguousarray(biases.T)                    # [128, 64]

    WiT = np.ascontiguousarray(gru_Wi.T)                       # [64, 192]
    shared = {
        "mem": memory,
        "w1h": w1h,
        "w1m": w1m,
        "w2t": w2t,
        # Wi rows duplicated: gi = [Wi;Wi] @ [msgA;msgB] (K=128)
        "wi2": np.ascontiguousarray(
            np.concatenate([WiT, WiT], axis=0).astype(bf)),    # [128, 192]
        "wh": np.ascontiguousarray(gru_Wh.T.astype(bf)),       # [64, 192]
        # [Wp1|Wp1]: act lands duplicated on partitions 0:64/64:128
        "wp1x": np.ascontiguousarray(
            np.concatenate([Wp1.T, Wp1.T], axis=1).astype(bf)),  # [64, 128]
        # Wp2T duplicated on partitions 0:64/64:128 for row-tiled pairs
        "wp2d": np.ascontiguousarray(
            np.concatenate([Wp2.T, Wp2.T], axis=0).astype(bf)),  # [128, 4096]
        "biases": biases,
        "ident": np.eye(128, dtype=f),
    }
    in_maps = []
    for c in range(NC):
        sl = slice(c * R, (c + 1) * R)
        odc_np = np.asarray(od_mat[sl], dtype=f)
        # odv[p, k*R+n] = od[c*R+n, k*128+p]
        odv = np.ascontiguousarray(
            odc_np.T.reshape(32, 128, R).transpose(1, 0, 2).reshape(128, 32 * R)
            .astype(e4))
        if USE_DMA_GATHER:
            # dma_gather idx layout: idx[(t*WL+j)*128+p] = walks[t*128+p, j],
            # wrap-16, replicated across the 8 Q7 core stripes
            wkc = walks[sl].reshape(NT, 128, WL).transpose(0, 2, 1).reshape(-1)
            widx = np.ascontiguousarray(np.tile(
                wkc.reshape(-1, 16).T.astype(np.int16), (8, 1)))  # [128, 256]
        else:
            # widx[p, t*WL+j] = walks[c*R + t*128 + p, j]
            widx = np.ascontiguousarray(
                walks[sl].reshape(NT, 128, WL).transpose(1, 0, 2)
                .reshape(128, NT * WL))
        memT = np.ascontiguousarray(memory[sl].T)
        in_maps.append(dict(
            shared,
            memT=np.ascontiguousarray(memT.astype(bf)),
            odv=odv,
            widx=widx,
        ))
    return in_maps


def _assemble(results):
    od = np.empty((N, N), dtype=np.float32)
    for c in range(NC):
        # outm[p, m, n] = od[c*R+n, m*128+p]
        od[c * R:(c + 1) * R, :] = (
            results[c]["outm"].astype(np.float32).transpose(2, 1, 0).reshape(R, N))
    return od


def _install_ntff_shim():
    """The agent image's antenv lacks axon_hooks, so trace=True dies on
    import. Recreate the module with the ctypes-based NTFF hook that
    trn_agent_boot would have registered."""
    import sys
    import types
    if "antenv.axon_hooks" in sys.modules:
        return
    from trn_agent_boot.trn_boot import _ntff_profile_via_ctypes
    hook = _ntff_profile_via_ctypes("/opt/axon/libaxon_pjrt.so")
    mod = types.ModuleType("antenv.axon_hooks")
    mod._hook = hook
    mod.get_axon_ntff_profile_hook = lambda: mod._hook
    mod.set_axon_ntff_profile_hook = lambda h: setattr(mod, "_hook", h)
    sys.modules["antenv.axon_hooks"] = mod


def run(inputs, trace=False):
    """Run on 8 NeuronCores; returns (od [N,N] f32, BassKernelResults)."""
    from concourse.bass_utils import run_bass_kernel_spmd
    if trace:
        try:
            _install_ntff_shim()
        except Exception as e:
            print(f"ntff shim failed ({e}); running without trace")
            trace = False
    nc = _get_program()
    in_maps = _host_prep(**inputs)
    res = run_bass_kernel_spmd(nc, in_maps, list(range(NC)), trace=trace)
    return _assemble(res.results), res


def kernel(**inputs):
    od, _ = run(inputs)
    return od


# revision 67
# speedup vs baseline: 1.1555x; 1.0004x over previous
"""Trainium2 Bass kernel for nn_DiscreteModel (GNN message passing).

Strategy: shard by node rows across 8 cores (512 rows each). All per-node
tensors are kept feature-major ([feature, node]) on-chip so the contraction
dim of every matmul sits on SBUF partitions. The host pre-transposes the
od_mat shard and all weights, folds the random-walk projection W_rw and the
1/8 mean into the layer-1 weight block, and pads HID 2112 -> 2176.

v3 (~125us, from the 158us v2). The PE issues a warm fp8-DoubleRow matmul
every ~216ns (2.4GHz; the 380ns trace "duration" is issue-to-drain), so the
od x W1 block floors at ~59us and everything else must hide behind it:
  head   : od loaded as 8 chunk tiles (2KB partition lines; per-chunk
           matmul gating, odc1 queued ahead of w1tB0 -> first DR matmul
           ~8.4us); w1 h-tiles split in halves across the sync/scalar
           queues; the w1m/w2t constant loads are split across slots
           5/6/7 so their backlog stays under the per-slot DMA slack;
           4 zero-DR warmup matmuls fill the PE until od chunk 0 lands.
  gather : 32 indirect DMAs (~1.1us of gpsimd SWDGE ucode each, ends
           ~56us; dma_gather would pay a ~13us Q7 ucode lib load first).
           gpsimd carries NOTHING else until the output DMAs.
  spill  : h<9 evacuate their od partial to SBUF (scalar ACT) and replay
           after the mixed k-tile exists (lag-2 finalize for h>=9, one
           replay per slot from slot 11, short drain; transposes emitted
           at slot 10, after the walk sums land ~60us). The walk-sum
           transposes read identx = ident + 0*odp[8] -- a data-dep pin so
           the Tile scheduler (sim-driven, reorders freely) cannot hoist
           them and their semaphore stalls into the early od stream.
  L2     : col-tiled pairs (even h -> psum[0:64], odd h -> [64:128],
           concurrent); halves combined for free via duplicated GRU Wi
           rows (gi = [Wi;Wi] @ [msgA;msgB], K=128 costs the same).
  relu   : alternates scalar ACT / vector add+max so neither engine paces
           the finalize chain; replay adds on vector, replay relu scalar.
  tail   : GRU memory operand in bf16 from the resident memT tile; wp1
           output M=128 with [Wp1|Wp1] so act lands duplicated on
           partitions 0:64/64:128; pred runs as 16 row-tiled PAIRS
           (tile_position (0,0)/(64,0), concurrent); evacuations alternate
           scalar/vector into bf16 staging; output DMA per 4 m-tiles on
           sync/gpsimd with [128, 32, R] DRAM layout (4KB lines).
fp8    : the od x W1 block (K=4096 of 4224) runs in fp8e4 DoubleRow mode.
         W1od is scaled x16 on host (relu(16x)=16relu(x); 1/16 folded
         into W2). Rel err ~1.24e-2 vs the 2e-2 gate.
Note: the axon TRN2 fleet drifts run-to-run (same NEFF 124..151us);
compare kernels only back-to-back within one window, min-of-3.
"""

import numpy as np

import concourse.bass as bass
import concourse.bacc as bacc
import concourse.tile as tile
from concourse import mybir

N = 4096        # nodes
MD = 64         # memory dim
MSG = 64        # message dim
WL = 8          # walk length
HID = 2112
HT = 17         # h-tiles (HID padded to 17*128 = 2176)
HIDP = HT * 128
NC = 8          # cores
R = N // NC     # rows (nodes) per core = 512
NT = R // 128   # node tiles per core = 4
F32 = mybir.dt.float32
F32R = mybir.dt.float32r
BF16 = mybir.dt.bfloat16
FP8 = mybir.dt.float8e4
I16 = mybir.dt.int16
I32 = mybir.dt.int32
WSCALE = 16.0   # W1 block scale so fp8 weights sit in e4m3 normal range
USE_DMA_GATHER = False   # Q7 SWDGE gather pays ~13us ucode lib load; the 32
                         # indirect DMAs (1.1us gpsimd ucode each) end sooner
N_WARM = 4      # zero DR matmuls fill the PE only until the first od
                # chunk lands (~8.7us); more would delay the real stream
LAG = 2         # h-tiles between od part and mixed finalize

_PROG = None


def _build_program():
    nc = bacc.Bacc("TRN2", target_bir_lowering=False, debug=False, num_devices=NC)

    # ---- DRAM I/O (all pre-laid-out on host, partition-major) ----
    mem_d = nc.dram_tensor("mem", [N, MD], F32, kind="ExternalInput").ap()
    memT_d = nc.dram_tensor("memT", [MD, R], BF16, kind="ExternalInput").ap()
    od_d = nc.dram_tensor("odv", [128, 32 * R], FP8, kind="ExternalInput").ap()
    if USE_DMA_GATHER:
        widx_d = nc.dram_tensor("widx", [128, NT * WL * 128 // 16], I16,
                                kind="ExternalInput").ap()
    else:
        widx32_d = nc.dram_tensor("widx", [128, NT * WL], I32,
                                  kind="ExternalInput").ap()
    w1h_d = nc.dram_tensor("w1h", [HT, 128, 32 * 128], FP8, kind="ExternalInput").ap()
    w1m_d = nc.dram_tensor("w1m", [128, HT * 128], BF16, kind="ExternalInput").ap()
    w2t_d = nc.dram_tensor("w2t", [128, HT * MSG], BF16, kind="ExternalInput").ap()
    wi2_d = nc.dram_tensor("wi2", [128, 3 * MD], BF16, kind="ExternalInput").ap()
    wh_d = nc.dram_tensor("wh", [MD, 3 * MD], BF16, kind="ExternalInput").ap()
    wp1_d = nc.dram_tensor("wp1x", [MD, 128], BF16, kind="ExternalInput").ap()
    wp2_d = nc.dram_tensor("wp2d", [128, N], BF16, kind="ExternalInput").ap()
    bias_d = nc.dram_tensor("biases", [128, 64], F32, kind="ExternalInput").ap()
    ident_d = nc.dram_tensor("ident", [128, 128], F32, kind="ExternalInput").ap()
    out_d = nc.dram_tensor("outm", [128, 32, R], BF16, kind="ExternalOutput").ap()

    AF = mybir.ActivationFunctionType
    DR = mybir.MatmulPerfMode.DoubleRow
    HK = 8 * 256            # half of a w1 h-tile (k-pairs 0..7)

    with tile.TileContext(nc) as tc:
        with (
            tc.tile_pool(name="consts", bufs=1) as consts,
            tc.tile_pool(name="w1p", bufs=3) as w1p,
            tc.tile_pool(name="gp", bufs=2) as gp,
            tc.tile_pool(name="hp", bufs=4) as hp,
            tc.tile_pool(name="gates", bufs=1) as gates,
            tc.tile_pool(name="ostg", bufs=3) as ostg,
            tc.tile_pool(name="pmm", bufs=5, space="PSUM") as pmm,
            tc.tile_pool(name="pacc", bufs=1, space="PSUM") as pacc,
            tc.tile_pool(name="ptr", bufs=1, space="PSUM") as ptr,
        ):
            # ---- walk indices first: gather feeds the mixed k-tile.
            # Split per node-tile so the first indirect DMA starts as soon
            # as its own 4KB of indices lands.
            if USE_DMA_GATHER:
                wk = consts.tile([128, NT * WL * 128 // 16], I16, tag="wk")
                nc.gpsimd.dma_start(out=wk[:], in_=widx_d[:])
            else:
                wk = consts.tile([128, NT * WL], I32, tag="wk")
                for t in range(NT):
                    nc.gpsimd.dma_start(out=wk[:, t * WL:(t + 1) * WL],
                                        in_=widx32_d[:, t * WL:(t + 1) * WL])

            # head DMA: the first DR matmul needs w1 h0 front half + od chunk
            # 0 only -> both lead their queues; od goes in 8 chunks of 2
            # k-pairs (2KB partition lines for full DMA efficiency), even on
            # sync, odd on scalar, so matmul k gates on chunk k//2.
            w1tA = [None] * HT
            w1tB = [None] * HT
            w1tA[0] = w1p.tile([128, HK], FP8, tag="w1tA", name="w1tA0")
            nc.sync.dma_start(out=w1tA[0][:], in_=w1h_d[0][:, :HK])
            odc = []
            for c in range(8):
                t = consts.tile([128, 4 * R], FP8, tag=f"odc{c}",
                                name=f"odc{c}")
                odc.append(t)
            nc.scalar.dma_start(out=odc[0][:], in_=od_d[:, 0:4 * R])
            nc.scalar.dma_start(out=odc[1][:], in_=od_d[:, 4 * R:8 * R])
            w1tB[0] = w1p.tile([128, HK], FP8, tag="w1tB", name="w1tB0")
            nc.scalar.dma_start(out=w1tB[0][:], in_=w1h_d[0][:, HK:])
            for c in range(2, 8):
                eng = nc.sync if c % 2 == 0 else nc.scalar
                eng.dma_start(out=odc[c][:],
                              in_=od_d[:, c * 4 * R:(c + 1) * 4 * R])
            for h in (1, 2):
                w1tA[h] = w1p.tile([128, HK], FP8, tag="w1tA", name=f"w1tA{h}")
                nc.sync.dma_start(out=w1tA[h][:], in_=w1h_d[h][:, :HK])
                w1tB[h] = w1p.tile([128, HK], FP8, tag="w1tB", name=f"w1tB{h}")
                nc.scalar.dma_start(out=w1tB[h][:], in_=w1h_d[h][:, HK:])

            # Q7 SWDGE gather for all 4096 walk rows:
            # gare[p, (t*WL+j)*MD : +MD] = mem[walks[t*128+p, j]]
            gare = consts.tile([128, NT * WL * MD], F32, tag="gare")
            if USE_DMA_GATHER:
                # >=2048 idxs per instruction hangs the Q7 ucode on HW;
                # 4x1024 (one per node-tile) costs ~1.2us SWDGE each
                for t in range(NT):
                    nc.gpsimd.dma_gather(
                        gare[:, t * WL * MD:(t + 1) * WL * MD].rearrange(
                            "p (g d) -> p g d", g=WL),
                        mem_d[:], wk[:, t * WL * 8:(t + 1) * WL * 8],
                        WL * 128, WL * 128, MD,
                    )
            else:
                for t in range(NT):
                    for j in range(WL):
                        o = (t * WL + j) * MD
                        nc.gpsimd.indirect_dma_start(
                            out=gare[:, o:o + MD],
                            out_offset=None,
                            in_=mem_d[:],
                            in_offset=bass.IndirectOffsetOnAxis(
                                ap=wk[:, t * WL + j:t * WL + j + 1], axis=0),
                        )

            # PE warmup: zero DR matmuls from ~6.5us pull HAM to full clock
            # before the real stream starts (idle >3.4us re-throttles)
            zx = consts.tile([128, 2 * R], FP8, tag="zx")
            nc.vector.memset(zx[:], 0)
            pdum = pmm.tile([128, R], F32, tag="mm")
            for _ in range(N_WARM):
                nc.tensor.matmul(
                    out=pdum[:],
                    lhsT=zx[:, 0:256].rearrange("p (two m) -> p two m", two=2),
                    rhs=zx[:].rearrange("p (two n) -> p two n", two=2),
                    start=True, stop=True, perf_mode=DR,
                )

            # identity from DRAM: gpsimd must stay free for the gather ucode
            ident = consts.tile([128, 128], F32, tag="ident")
            nc.scalar.dma_start(out=ident[:], in_=ident_d[:])
            biasp = consts.tile([128, 64], F32, tag="biasp")
            nc.scalar.dma_start(out=biasp[:], in_=bias_d[:])

            # mixed rawT k-tile: [0:64] = memT shard, [64:128] = GsT (walk sums)
            mixed = consts.tile([128, R], BF16, tag="mixed")
            nc.scalar.dma_start(out=mixed[0:MD, :], in_=memT_d[:])

            # constant tiles; their DMAs are emitted inside the h-loop so
            # they queue behind the od/w1 head flood (needed ~55us onward)
            w1m_sb = consts.tile([128, HT * 128], BF16, tag="w1m")
            w2t_sb = consts.tile([128, HT * MSG], BF16, tag="w2t")
            wh_sb = consts.tile([MD, 3 * MD], BF16, tag="wh")
            wi2_sb = consts.tile([128, 3 * MD], BF16, tag="wi2")
            wp1_sb = consts.tile([MD, 128], BF16, tag="wp1x")
            wp2_sb = consts.tile([128, N], BF16, tag="wp2d")

            # preload the sigmoid/tanh ACT table while the head is DMA-paced
            # (otherwise a 1.28us ACT_TABLE_LOAD lands on the GRU chain)
            warm = gates.tile([MD, 4], F32, tag="warm")
            nc.scalar.activation(warm[:, 0:2], biasp[0:MD, 0:2], AF.Sigmoid)
            nc.scalar.activation(warm[:, 2:4], biasp[0:MD, 0:2], AF.Tanh)

            # walk sums on DVE (gated on the gather), one per node-tile
            m1s = [None] * NT
            for t in range(NT):
                ga3 = gare[:, t * WL * MD:(t + 1) * WL * MD].rearrange(
                    "p (j d) -> p j d", j=WL)
                m4 = gp.tile([128, 4 * MD], F32, tag="m4")
                m43 = m4[:].rearrange("p (j d) -> p j d", j=4)
                nc.vector.tensor_add(out=m43, in0=ga3[:, 0:4, :], in1=ga3[:, 4:8, :])
                m2 = gp.tile([128, 2 * MD], F32, tag="m2")
                m23 = m2[:].rearrange("p (j d) -> p j d", j=2)
                nc.vector.tensor_add(out=m23, in0=m43[:, 0:2, :], in1=m43[:, 2:4, :])
                m1t = gp.tile([128, MD], F32, tag=f"m1_{t}")
                nc.vector.tensor_add(out=m1t[:], in0=m2[:, 0:MD],
                                     in1=m2[:, MD:2 * MD])
                m1s[t] = m1t

            mixed_r = mixed[:]

            # ---- layer 1 (fp8 DoubleRow); the gather (4x ~8.6us Q7 ucode)
            # only completes ~44us in, so h < SPILL spill their od partial to
            # SBUF (freeing the PSUM bank) and replay one per slot once the
            # mixed tile exists; h >= SPILL run a lag-2 finalize.
            SPILL = 9
            psL2 = pacc.tile([128, R], F32, tag="l2")
            odp = consts.tile([128, SPILL * R], F32, tag="odp")
            identx = consts.tile([128, 128], F32, tag="identx")
            pss = {}
            hids = {}
            l2n = [0, 0]
            L2N = [9, 8]   # even/odd L2 stream lengths

            def emit_l2(h):
                half = h % 2
                nc.tensor.matmul(
                    out=psL2[half * 64:(half + 1) * 64, :],
                    lhsT=w2t_sb[:, h * MSG:(h + 1) * MSG],
                    rhs=hids.pop(h)[:],
                    start=(l2n[half] == 0), stop=(l2n[half] == L2N[half] - 1),
                )
                l2n[half] += 1

            def emit_relu(h, src):
                # alternate relu between scalar ACT and vector (add-bias,
                # max 0) so neither engine paces the finalize/replay chain
                hid = hp.tile([128, R], BF16, tag="hid")
                if h % 2 == 0:
                    nc.scalar.activation(hid[:], src, AF.Relu,
                                         bias=biasp[:, h:h + 1])
                else:
                    nc.vector.tensor_scalar(
                        out=hid[:], in0=src, scalar1=biasp[:, h:h + 1],
                        scalar2=0.0, op0=mybir.AluOpType.add,
                        op1=mybir.AluOpType.max)
                hids[h] = hid

            def finalize(h):
                ps = pss.pop(h)
                nc.tensor.matmul(
                    out=ps[:], lhsT=w1m_sb[:, h * 128:(h + 1) * 128],
                    rhs=mixed_r, start=False, stop=True,
                )
                emit_relu(h, ps[:])
                emit_l2(h)

            def replay(h):
                ps = pmm.tile([128, R], F32, tag="mm")
                nc.tensor.matmul(
                    out=ps[:], lhsT=w1m_sb[:, h * 128:(h + 1) * 128],
                    rhs=mixed_r, start=True, stop=True,
                )
                pre = gp.tile([128, R], F32, tag="clt")
                nc.vector.tensor_add(out=pre[:], in0=ps[:],
                                     in1=odp[:, h * R:(h + 1) * R])
                # vector already carries the add: replay relu goes to scalar
                hid = hp.tile([128, R], BF16, tag="hid")
                nc.scalar.activation(hid[:], pre[:], AF.Relu,
                                     bias=biasp[:, h:h + 1])
                hids[h] = hid
                emit_l2(h)

            for h in range(HT):
                if h >= 3:
                    w1tA[h] = w1p.tile([128, HK], FP8, tag="w1tA",
                                       name=f"w1tA{h}")
                    w1tB[h] = w1p.tile([128, HK], FP8, tag="w1tB",
                                       name=f"w1tB{h}")
                    engA = nc.sync if h % 2 == 1 else nc.scalar
                    engB = nc.scalar if h % 2 == 1 else nc.sync
                    engA.dma_start(out=w1tA[h][:], in_=w1h_d[h][:, :HK])
                    engB.dma_start(out=w1tB[h][:], in_=w1h_d[h][:, HK:])
                ps = pmm.tile([128, R], F32, tag="mm")
                for k in range(16):
                    wt = w1tA[h] if k < 8 else w1tB[h]
                    nc.tensor.matmul(
                        out=ps[:],
                        lhsT=wt[:, (k % 8) * 256:(k % 8 + 1) * 256].rearrange(
                            "p (two m) -> p two m", two=2),
                        rhs=odc[k // 2][:, (k % 2) * 2 * R:(k % 2 + 1) * 2 * R]
                        .rearrange("p (two n) -> p two n", two=2),
                        start=(k == 0), stop=(h < SPILL),
                        perf_mode=DR,
                    )
                if h < SPILL:
                    # evacuate pre-activation od partial (scalar; the vector
                    # stream is held by the gather-gated walk sums)
                    nc.scalar.activation(odp[:, h * R:(h + 1) * R], ps[:],
                                         AF.Identity)
                else:
                    pss[h] = ps
                if h == SPILL - 1:
                    # identx = ident + 0*odp[h]: a scheduler pin — the mix
                    # transposes read identx, so no schedule can hoist them
                    # (and their semaphore stalls) into the early od stream
                    tmpid = gp.tile([128, 128], F32, tag="tmpid")
                    nc.vector.tensor_scalar_mul(
                        out=tmpid[:], in0=odp[:, h * R:h * R + 128],
                        scalar1=0.0)
                    nc.vector.tensor_add(out=identx[:], in0=ident[:],
                                         in1=tmpid[:])
                if h == 5:
                    # replays (h<9) read the low half first
                    nc.scalar.dma_start(out=w1m_sb[:, :9 * 128],
                                        in_=w1m_d[:, :9 * 128])
                if h == 6:
                    nc.sync.dma_start(out=w2t_sb[:], in_=w2t_d[:])
                if h == 7:
                    nc.scalar.dma_start(out=w1m_sb[:, 9 * 128:],
                                        in_=w1m_d[:, 9 * 128:])
                if h == 12:
                    # tail-only constants: late so they never delay the w1
                    # stream that gates the od matmuls
                    nc.scalar.dma_start(out=wh_sb[:], in_=wh_d[:])
                    nc.scalar.dma_start(out=wi2_sb[:], in_=wi2_d[:])
                    nc.scalar.dma_start(out=wp1_sb[:], in_=wp1_d[:])
                    nc.sync.dma_start(out=wp2_sb[:], in_=wp2_d[:])
                if h == 14:
                    # h_n = memT @ Wh_n + bias depends only on memT; do it in
                    # the ramp where the PE has slack
                    ps_hn = pmm.tile([MD, R], F32, tag="mm")
                    nc.tensor.matmul(out=ps_hn[:], lhsT=wh_sb[:, 128:192],
                                     rhs=mixed[0:MD, :], start=True, stop=True)
                    hnb = gates.tile([MD, R], F32, tag="hnb")
                    nc.vector.tensor_scalar_add(out=hnb[:], in0=ps_hn[:],
                                                scalar1=biasp[0:MD, 20:21])
                if h == SPILL + 1:
                    # GsT transposes; PE reaches them ~62us, sums done ~60us
                    for t in range(NT):
                        tr = ptr.tile([MD, 128], F32, tag="tr")
                        nc.tensor.transpose(out=tr[:], in_=m1s[t][:],
                                            identity=identx[:])
                        nc.vector.tensor_copy(
                            out=mixed[MD:128, t * 128:(t + 1) * 128], in_=tr[:])
                if h >= SPILL + LAG:
                    finalize(h - LAG)
                    replay(h - SPILL - LAG)
            finalize(HT - 2)
            finalize(HT - 1)
            for h in range(min(HT - SPILL - LAG, SPILL), SPILL):
                replay(h)

            # msg (pre-b2, which is folded into the GRU input bias):
            # both L2 col halves evacuated in one op; the GRU adds them by
            # using duplicated Wi rows (K=128 costs the same as K=64)
            msg2x = gates.tile([128, R], BF16, tag="msg2x")
            nc.scalar.activation(msg2x[:], psL2[:], AF.Identity)
            msg_r = msg2x[:]
            memT_r = mixed[0:MD, :]

            # ---- GRU + prediction, column-split so the serial ACT/DVE chain
            #      pipelines across halves and the PE never idles >3.4us.
            ps_r = pmm.tile([MD, R], F32, tag="mm")
            nc.tensor.matmul(out=ps_r[:], lhsT=wi2_sb[:, 0:MD], rhs=msg_r,
                             start=True, stop=False)
            nc.tensor.matmul(out=ps_r[:], lhsT=wh_sb[:, 0:MD], rhs=memT_r,
                             start=False, stop=True)
            ps_z = pmm.tile([MD, R], F32, tag="mm")
            nc.tensor.matmul(out=ps_z[:], lhsT=wi2_sb[:, MD:128], rhs=msg_r,
                             start=True, stop=False)
            nc.tensor.matmul(out=ps_z[:], lhsT=wh_sb[:, MD:128], rhs=memT_r,
                             start=False, stop=True)
            ps_in = pmm.tile([MD, R], F32, tag="mm")
            nc.tensor.matmul(out=ps_in[:], lhsT=wi2_sb[:, 128:192], rhs=msg_r,
                             start=True, stop=True)
            r_t = gates.tile([MD, R], F32, tag="r_t")
            z_t = gates.tile([MD, R], F32, tag="z_t")
            rhn = gates.tile([MD, R], F32, tag="rhn")
            npre = gates.tile([MD, R], F32, tag="npre")
            n_t = gates.tile([MD, R], F32, tag="n_t")
            zc_t = gates.tile([MD, R], F32, tag="zc_t")
            zm_t = gates.tile([MD, R], BF16, tag="zm_t")
            ncz = gates.tile([MD, R], BF16, tag="ncz")
            ps_pred = pacc.tile([128, R], F32, tag="pred")
            act2 = gates.tile([128, R], BF16, tag="act2")
            HR = R // 2
            for x in range(2):
                cs = slice(x * HR, (x + 1) * HR)
                nc.scalar.activation(r_t[:, cs], ps_r[:, cs], AF.Sigmoid,
                                     bias=biasp[0:MD, 17:18])
                nc.scalar.activation(z_t[:, cs], ps_z[:, cs], AF.Sigmoid,
                                     bias=biasp[0:MD, 18:19])
                # upd = (1-z)*n + z*mem = zc*n + zm; zc/zm go on vector right
                # after the z sigmoid (gpsimd would pay a Q7 ucode lib swap
                # after the gathers that serializes the whole chain)
                nc.vector.tensor_scalar(out=zc_t[:, cs], in0=z_t[:, cs],
                                        scalar1=-1.0, scalar2=1.0,
                                        op0=mybir.AluOpType.mult,
                                        op1=mybir.AluOpType.add)
                nc.vector.tensor_mul(out=zm_t[:, cs], in0=z_t[:, cs],
                                     in1=memT_r[:, cs])
                # upd = ncz + zm is absorbed into wp1 by linearity:
                # Wp1@(ncz+zm) = Wp1@ncz + Wp1@zm. The zm part runs right
                # after the z sigmoid, off the r->tanh critical chain
                nc.tensor.matmul(out=ps_pred[:, cs], lhsT=wp1_sb[:],
                                 rhs=zm_t[:, cs], start=True, stop=False)
                nc.vector.tensor_mul(out=rhn[:, cs], in0=r_t[:, cs], in1=hnb[:, cs])
                nc.vector.tensor_add(out=npre[:, cs], in0=ps_in[:, cs], in1=rhn[:, cs])
                nc.scalar.activation(n_t[:, cs], npre[:, cs], AF.Tanh,
                                     bias=biasp[0:MD, 19:20])
                nc.vector.tensor_mul(out=ncz[:, cs], in0=zc_t[:, cs], in1=n_t[:, cs])
                # [Wp1|Wp1] -> act duplicated on partitions 0:64 / 64:128 so
                # pred pairs can row-tile
                nc.tensor.matmul(out=ps_pred[:, cs], lhsT=wp1_sb[:], rhs=ncz[:, cs],
                                 start=False, stop=True)
                nc.scalar.activation(act2[:, cs], ps_pred[:, cs], AF.Relu,
                                     bias=biasp[:, 21:22])

            # ---- prediction m-loop: 16 row-tiled PAIRS (tile_position
            #      (0,0)/(64,0), concurrent on the PE); evacuations alternate
            #      scalar/vector; output staged bf16, 0.5MB DMAs
            GRP = 4
            for m2 in range(16):
                m0, m1 = 2 * m2, 2 * m2 + 1
                psA = pmm.tile([128, R], F32, tag="mm")
                psB = pmm.tile([128, R], F32, tag="mm")
                nc.tensor.matmul(out=psA[:],
                                 lhsT=wp2_sb[0:64, m0 * 128:(m0 + 1) * 128],
                                 rhs=act2[0:64, :], start=True, stop=True)
                nc.tensor.matmul(out=psB[:],
                                 lhsT=wp2_sb[64:128, m1 * 128:(m1 + 1) * 128],
                                 rhs=act2[64:128, :], start=True, stop=True)
                if m0 % GRP == 0:
                    stage = ostg.tile([128, GRP * R], BF16, tag="stage")
                slA = stage[:, (m0 % GRP) * R:(m0 % GRP + 1) * R]
                slB = stage[:, (m1 % GRP) * R:(m1 % GRP + 1) * R]
                nc.scalar.activation(slA, psA[:], AF.Identity,
                                     bias=biasp[:, 22 + m0:23 + m0])
                nc.vector.tensor_scalar_add(out=slB, in0=psB[:],
                                            scalar1=biasp[:, 22 + m1:23 + m1])
                if m1 % GRP == GRP - 1:
                    g = m1 // GRP
                    if g >= 6:
                        # the last transfers gate the end-of-kernel drain:
                        # split them in halves across both free queues
                        st3 = stage[:].rearrange("p (g n) -> p g n", g=GRP)
                        nc.sync.dma_start(
                            out=out_d[:, g * GRP:g * GRP + 2, :],
                            in_=st3[:, 0:2])
                        nc.gpsimd.dma_start(
                            out=out_d[:, g * GRP + 2:(g + 1) * GRP, :],
                            in_=st3[:, 2:4])
                    else:
                        oeng = nc.sync if g % 2 == 0 else nc.gpsimd
                        oeng.dma_start(
                            out=out_d[:, g * GRP:(g + 1) * GRP, :],
                            in_=stage[:].rearrange("p (g n) -> p g n", g=GRP))

    nc.compile()
    return nc


def _get_program():
    global _PROG
    if _PROG is None:
        _PROG = _build_program()
    return _PROG


def _host_prep(memory, od_mat, walks, W_rw, b_rw, W1, b1, W2, b2,
               gru_Wi, gru_bi, gru_Wh, gru_bh, Wp1, bp1, Wp2, bp2):
    import ml_dtypes
    f = np.float32
    bf = ml_dtypes.bfloat16
    e4 = ml_dtypes.float8_e4m3fn
    memory = np.ascontiguousarray(np.asarray(memory), dtype=f)
    od_mat = np.asarray(od_mat)
    walks = np.asarray(walks).astype(np.int32)
    W_rw = np.asarray(W_rw, dtype=f); b_rw = np.asarray(b_rw, dtype=f)
    W1 = np.asarray(W1, dtype=f); b1 = np.asarray(b1, dtype=f)
    W2 = np.asarray(W2, dtype=f); b2 = np.asarray(b2, dtype=f)
    gru_Wi = np.asarray(gru_Wi, dtype=f); gru_bi = np.asarray(gru_bi, dtype=f)
    gru_Wh = np.asarray(gru_Wh, dtype=f); gru_bh = np.asarray(gru_bh, dtype=f)
    Wp1 = np.asarray(Wp1, dtype=f); bp1 = np.asarray(bp1, dtype=f)
    Wp2 = np.asarray(Wp2, dtype=f); bp2 = np.asarray(bp2, dtype=f)

    # layer-1 weights, column-permuted to [od | dest | walk] with W_rw and the
    # 1/8 mean folded into the walk block; HID padded to 2176; whole block
    # scaled x16 so the fp8 od weights sit in e4m3 normal range (1/16 folded
    # into W2; exact since relu(16x)=16relu(x))
    W1od = W1[:, MD:MD + N]
    W1dest = W1[:, 0:MD]
    W1rw = W1[:, MD + N:]
    W1g = (W1rw @ W_rw) / np.float32(8.0)
    W1p = np.concatenate([W1od, W1dest, W1g], axis=1) * np.float32(WSCALE)
    W1pT = np.zeros((33 * 128, HIDP), dtype=f)
    W1pT[:, :HID] = W1p.T
    # w1h[h][p, k*128+c] = W1pT[k*128+p, h*128+c] for the 32 od k-tiles
    # (pairs of adjacent k-tiles feed one DoubleRow matmul);
    # the mixed k-tile (rows 4096:4224) is its own resident tensor w1m
    w1h = np.ascontiguousarray(
        W1pT[:32 * 128].reshape(32, 128, HT, 128)
        .transpose(2, 1, 0, 3).reshape(HT, 128, 32 * 128).astype(e4))
    w1m = np.ascontiguousarray(W1pT[32 * 128:].astype(bf))  # [128, 2176]

    b1p = np.zeros(HIDP, dtype=f)
    b1p[:HID] = (b1 + W1rw @ b_rw) * np.float32(WSCALE)

    W2tp = np.zeros((HIDP, MSG), dtype=f)
    W2tp[:HID] = W2.T / np.float32(WSCALE)
    # w2t[p, h*64+c] = W2tp[h*128+p, c]
    w2t = np.ascontiguousarray(
        W2tp.reshape(HT, 128, MSG).transpose(1, 0, 2).reshape(128, HT * MSG)
        .astype(bf))

    def pad128(v):
        o = np.zeros(128, dtype=f)
        o[:v.shape[0]] = v
        return o

    # b2 folded through the GRU input weights: gi = Wi@(msg'+b2)+bi
    gbi_f = gru_bi + gru_Wi @ b2

    # biases packed as [128 partitions, 64 columns]
    biases = np.zeros((64, 128), dtype=f)
    biases[0:HT] = b1p.reshape(HT, 128)
    grz = gbi_f[:128] + gru_bh[:128]
    biases[17] = pad128(grz[:64])      # r gate bias
    biases[18] = pad128(grz[64:])      # z gate bias
    biases[19] = pad128(gbi_f[128:])
    biases[20] = pad128(gru_bh[128:])
    biases[21] = np.concatenate([bp1, bp1])  # duplicated for act2 row-tiling
    biases[22:54] = bp2.reshape(32, 128)
    biases = np.ascontiguousarray(biases.T)                    # [128, 64]

    WiT = np.ascontiguousarray(gru_Wi.T)                       # [64, 192]
    shared = {
        "mem": memory,
        "w1h": w1h,
        "w1m": w1m,
        "w2t": w2t,
        # Wi rows duplicated: gi = [Wi;Wi] @ [msgA;msgB] (K=128)
        "wi2": np.ascontiguousarray(
            np.concatenate([WiT, WiT], axis=0).astype(bf)),    # [128, 192]
        "wh": np.ascontiguousarray(gru_Wh.T.astype(bf)),       # [64, 192]
        # [Wp1|Wp1]: act lands duplicated on partitions 0:64/64:128
        "wp1x": np.ascontiguousarray(
            np.concatenate([Wp1.T, Wp1.T], axis=1).astype(bf)),  # [64, 128]
        # Wp2T duplicated on partitions 0:64/64:128 for row-tiled pairs
        "wp2d": np.ascontiguousarray(
            np.concatenate([Wp2.T, Wp2.T], axis=0).astype(bf)),  # [128, 4096]
        "biases": biases,
        "ident": np.eye(128, dtype=f),
    }
    in_maps = []
    for c in range(NC):
        sl = slice(c * R, (c + 1) * R)
        odc_np = np.asarray(od_mat[sl], dtype=f)
        # odv[p, k*R+n] = od[c*R+n, k*128+p]
        odv = np.ascontiguousarray(
            odc_np.T.reshape(32, 128, R).transpose(1, 0, 2).reshape(128, 32 * R)
            .astype(e4))
        if USE_DMA_GATHER:
            # dma_gather idx layout: idx[(t*WL+j)*128+p] = walks[t*128+p, j],
            # wrap-16, replicated across the 8 Q7 core stripes
            wkc = walks[sl].reshape(NT, 128, WL).transpose(0, 2, 1).reshape(-1)
            widx = np.ascontiguousarray(np.tile(
                wkc.reshape(-1, 16).T.astype(np.int16), (8, 1)))  # [128, 256]
        else:
            # widx[p, t*WL+j] = walks[c*R + t*128 + p, j]
            widx = np.ascontiguousarray(
                walks[sl].reshape(NT, 128, WL).transpose(1, 0, 2)
                .reshape(128, NT * WL))
        memT = np.ascontiguousarray(memory[sl].T)
        in_maps.append(dict(
            shared,
            memT=np.ascontiguousarray(memT.astype(bf)),
            odv=odv,
            widx=widx,
        ))
    return in_maps


def _assemble(results):
    od = np.empty((N, N), dtype=np.float32)
    for c in range(NC):
        # outm[p, m, n] = od[c*R+n, m*128+p]
        od[c * R:(c + 1) * R, :] = (
            results[c]["outm"].astype(np.float32).transpose(2, 1, 0).reshape(R, N))
    return od


def _install_ntff_shim():
    """The agent image's antenv lacks axon_hooks, so trace=True dies on
    import. Recreate the module with the ctypes-based NTFF hook that
    trn_agent_boot would have registered."""
    import sys
    import types
    if "antenv.axon_hooks" in sys.modules:
        return
    from trn_agent_boot.trn_boot import _ntff_profile_via_ctypes
    hook = _ntff_profile_via_ctypes("/opt/axon/libaxon_pjrt.so")
    mod = types.ModuleType("antenv.axon_hooks")
    mod._hook = hook
    mod.get_axon_ntff_profile_hook = lambda: mod._hook
    mod.set_axon_ntff_profile_hook = lambda h: setattr(mod, "_hook", h)
    sys.modules["antenv.axon_hooks"] = mod


def run(inputs, trace=False):
    """Run on 8 NeuronCores; returns (od [N,N] f32, BassKernelResults)."""
    from concourse.bass_utils import run_bass_kernel_spmd
    if trace:
        try:
            _install_ntff_shim()
        except Exception as e:
            print(f"ntff shim failed ({e}); running without trace")
            trace = False
    nc = _get_program()
    in_maps = _host_prep(**inputs)
    res = run_bass_kernel_spmd(nc, in_maps, list(range(NC)), trace=trace)
    return _assemble(res.results), res


def kernel(**inputs):
    od, _ = run(inputs)
    return od


# revision 68
# speedup vs baseline: 1.1635x; 1.0070x over previous
"""Trainium2 Bass kernel for nn_DiscreteModel (GNN message passing).

Strategy: shard by node rows across 8 cores (512 rows each). All per-node
tensors are kept feature-major ([feature, node]) on-chip so the contraction
dim of every matmul sits on SBUF partitions. The host pre-transposes the
od_mat shard and all weights, folds the random-walk projection W_rw and the
1/8 mean into the layer-1 weight block, and pads HID 2112 -> 2176.

v3 (~125us, from the 158us v2). The PE issues a warm fp8-DoubleRow matmul
every ~216ns (2.4GHz; the 380ns trace "duration" is issue-to-drain), so the
od x W1 block floors at ~59us and everything else must hide behind it:
  head   : od loaded as 8 chunk tiles (2KB partition lines; per-chunk
           matmul gating, odc1 queued ahead of w1tB0 -> first DR matmul
           ~8.4us); w1 h-tiles split in halves across the sync/scalar
           queues; the w1m/w2t constant loads are split across slots
           5/6/7 so their backlog stays under the per-slot DMA slack;
           4 zero-DR warmup matmuls fill the PE until od chunk 0 lands.
  gather : 32 indirect DMAs (~1.1us of gpsimd SWDGE ucode each, ends
           ~56us; dma_gather would pay a ~13us Q7 ucode lib load first).
           gpsimd carries NOTHING else until the output DMAs.
  spill  : h<9 evacuate their od partial to SBUF (scalar ACT) and replay
           after the mixed k-tile exists (lag-2 finalize for h>=9, one
           replay per slot from slot 11, short drain; transposes emitted
           at slot 10, after the walk sums land ~60us). The walk-sum
           transposes read identx = ident + 0*odp[8] -- a data-dep pin so
           the Tile scheduler (sim-driven, reorders freely) cannot hoist
           them and their semaphore stalls into the early od stream.
  L2     : col-tiled pairs (even h -> psum[0:64], odd h -> [64:128],
           concurrent); halves combined for free via duplicated GRU Wi
           rows (gi = [Wi;Wi] @ [msgA;msgB], K=128 costs the same).
  relu   : alternates scalar ACT / vector add+max so neither engine paces
           the finalize chain; replay adds on vector, replay relu scalar.
  tail   : GRU memory operand in bf16 from the resident memT tile; wp1
           output M=128 with [Wp1|Wp1] so act lands duplicated on
           partitions 0:64/64:128; pred runs as 16 row-tiled PAIRS
           (tile_position (0,0)/(64,0), concurrent); evacuations alternate
           scalar/vector into bf16 staging; output DMA per 4 m-tiles on
           sync/gpsimd with [128, 32, R] DRAM layout (4KB lines).
fp8    : the od x W1 block (K=4096 of 4224) runs in fp8e4 DoubleRow mode.
         W1od is scaled x16 on host (relu(16x)=16relu(x); 1/16 folded
         into W2). Rel err ~1.24e-2 vs the 2e-2 gate.
Note: the axon TRN2 fleet drifts run-to-run (same NEFF 124..151us);
compare kernels only back-to-back within one window, min-of-3.
"""

import numpy as np

import concourse.bass as bass
import concourse.bacc as bacc
import concourse.tile as tile
from concourse import mybir

N = 4096        # nodes
MD = 64         # memory dim
MSG = 64        # message dim
WL = 8          # walk length
HID = 2112
HT = 17         # h-tiles (HID padded to 17*128 = 2176)
HIDP = HT * 128
NC = 8          # cores
R = N // NC     # rows (nodes) per core = 512
NT = R // 128   # node tiles per core = 4
F32 = mybir.dt.float32
F32R = mybir.dt.float32r
BF16 = mybir.dt.bfloat16
FP8 = mybir.dt.float8e4
I16 = mybir.dt.int16
I32 = mybir.dt.int32
WSCALE = 16.0   # W1 block scale so fp8 weights sit in e4m3 normal range
USE_DMA_GATHER = False   # Q7 SWDGE gather pays ~13us ucode lib load; the 32
                         # indirect DMAs (1.1us gpsimd ucode each) end sooner
N_WARM = 4      # zero DR matmuls fill the PE only until the first od
                # chunk lands (~8.7us); more would delay the real stream
LAG = 2         # h-tiles between od part and mixed finalize

_PROG = None


def _build_program():
    nc = bacc.Bacc("TRN2", target_bir_lowering=False, debug=False, num_devices=NC)

    # ---- DRAM I/O (all pre-laid-out on host, partition-major) ----
    mem_d = nc.dram_tensor("mem", [N, MD], F32, kind="ExternalInput").ap()
    memT_d = nc.dram_tensor("memT", [MD, R], BF16, kind="ExternalInput").ap()
    od_d = nc.dram_tensor("odv", [128, 32 * R], FP8, kind="ExternalInput").ap()
    if USE_DMA_GATHER:
        widx_d = nc.dram_tensor("widx", [128, NT * WL * 128 // 16], I16,
                                kind="ExternalInput").ap()
    else:
        widx32_d = nc.dram_tensor("widx", [128, NT * WL], I32,
                                  kind="ExternalInput").ap()
    w1h_d = nc.dram_tensor("w1h", [HT, 128, 32 * 128], FP8, kind="ExternalInput").ap()
    w1m_d = nc.dram_tensor("w1m", [128, HT * 128], BF16, kind="ExternalInput").ap()
    w2t_d = nc.dram_tensor("w2t", [128, HT * MSG], BF16, kind="ExternalInput").ap()
    wi2_d = nc.dram_tensor("wi2", [128, 3 * MD], BF16, kind="ExternalInput").ap()
    wh_d = nc.dram_tensor("wh", [MD, 3 * MD], BF16, kind="ExternalInput").ap()
    wp1_d = nc.dram_tensor("wp1x", [MD, 128], BF16, kind="ExternalInput").ap()
    wp2_d = nc.dram_tensor("wp2d", [128, N], BF16, kind="ExternalInput").ap()
    bias_d = nc.dram_tensor("biases", [128, 64], F32, kind="ExternalInput").ap()
    ident_d = nc.dram_tensor("ident", [128, 128], F32, kind="ExternalInput").ap()
    out_d = nc.dram_tensor("outm", [128, 32, R], BF16, kind="ExternalOutput").ap()

    AF = mybir.ActivationFunctionType
    DR = mybir.MatmulPerfMode.DoubleRow
    HK = 8 * 256            # half of a w1 h-tile (k-pairs 0..7)

    with tile.TileContext(nc) as tc:
        with (
            tc.tile_pool(name="consts", bufs=1) as consts,
            tc.tile_pool(name="w1p", bufs=3) as w1p,
            tc.tile_pool(name="gp", bufs=2) as gp,
            tc.tile_pool(name="hp", bufs=4) as hp,
            tc.tile_pool(name="gates", bufs=1) as gates,
            tc.tile_pool(name="ostg", bufs=4) as ostg,
            tc.tile_pool(name="pmm", bufs=5, space="PSUM") as pmm,
            tc.tile_pool(name="pacc", bufs=1, space="PSUM") as pacc,
            tc.tile_pool(name="ptr", bufs=1, space="PSUM") as ptr,
        ):
            # ---- walk indices first: gather feeds the mixed k-tile.
            # Split per node-tile so the first indirect DMA starts as soon
            # as its own 4KB of indices lands.
            if USE_DMA_GATHER:
                wk = consts.tile([128, NT * WL * 128 // 16], I16, tag="wk")
                nc.gpsimd.dma_start(out=wk[:], in_=widx_d[:])
            else:
                wk = consts.tile([128, NT * WL], I32, tag="wk")
                for t in range(NT):
                    nc.gpsimd.dma_start(out=wk[:, t * WL:(t + 1) * WL],
                                        in_=widx32_d[:, t * WL:(t + 1) * WL])

            # head DMA: the first DR matmul needs w1 h0 front half + od chunk
            # 0 only -> both lead their queues; od goes in 8 chunks of 2
            # k-pairs (2KB partition lines for full DMA efficiency), even on
            # sync, odd on scalar, so matmul k gates on chunk k//2.
            w1tA = [None] * HT
            w1tB = [None] * HT
            w1tA[0] = w1p.tile([128, HK], FP8, tag="w1tA", name="w1tA0")
            nc.sync.dma_start(out=w1tA[0][:], in_=w1h_d[0][:, :HK])
            odc = []
            for c in range(8):
                t = consts.tile([128, 4 * R], FP8, tag=f"odc{c}",
                                name=f"odc{c}")
                odc.append(t)
            nc.scalar.dma_start(out=odc[0][:], in_=od_d[:, 0:4 * R])
            nc.scalar.dma_start(out=odc[1][:], in_=od_d[:, 4 * R:8 * R])
            w1tB[0] = w1p.tile([128, HK], FP8, tag="w1tB", name="w1tB0")
            nc.scalar.dma_start(out=w1tB[0][:], in_=w1h_d[0][:, HK:])
            for c in range(2, 8):
                eng = nc.sync if c % 2 == 0 else nc.scalar
                eng.dma_start(out=odc[c][:],
                              in_=od_d[:, c * 4 * R:(c + 1) * 4 * R])
            for h in (1, 2):
                w1tA[h] = w1p.tile([128, HK], FP8, tag="w1tA", name=f"w1tA{h}")
                nc.sync.dma_start(out=w1tA[h][:], in_=w1h_d[h][:, :HK])
                w1tB[h] = w1p.tile([128, HK], FP8, tag="w1tB", name=f"w1tB{h}")
                nc.scalar.dma_start(out=w1tB[h][:], in_=w1h_d[h][:, HK:])

            # Q7 SWDGE gather for all 4096 walk rows:
            # gare[p, (t*WL+j)*MD : +MD] = mem[walks[t*128+p, j]]
            gare = consts.tile([128, NT * WL * MD], F32, tag="gare")
            if USE_DMA_GATHER:
                # >=2048 idxs per instruction hangs the Q7 ucode on HW;
                # 4x1024 (one per node-tile) costs ~1.2us SWDGE each
                for t in range(NT):
                    nc.gpsimd.dma_gather(
                        gare[:, t * WL * MD:(t + 1) * WL * MD].rearrange(
                            "p (g d) -> p g d", g=WL),
                        mem_d[:], wk[:, t * WL * 8:(t + 1) * WL * 8],
                        WL * 128, WL * 128, MD,
                    )
            else:
                for t in range(NT):
                    for j in range(WL):
                        o = (t * WL + j) * MD
                        nc.gpsimd.indirect_dma_start(
                            out=gare[:, o:o + MD],
                            out_offset=None,
                            in_=mem_d[:],
                            in_offset=bass.IndirectOffsetOnAxis(
                                ap=wk[:, t * WL + j:t * WL + j + 1], axis=0),
                        )

            # PE warmup: zero DR matmuls from ~6.5us pull HAM to full clock
            # before the real stream starts (idle >3.4us re-throttles)
            zx = consts.tile([128, 2 * R], FP8, tag="zx")
            nc.vector.memset(zx[:], 0)
            pdum = pmm.tile([128, R], F32, tag="mm")
            for _ in range(N_WARM):
                nc.tensor.matmul(
                    out=pdum[:],
                    lhsT=zx[:, 0:256].rearrange("p (two m) -> p two m", two=2),
                    rhs=zx[:].rearrange("p (two n) -> p two n", two=2),
                    start=True, stop=True, perf_mode=DR,
                )

            # identity from DRAM: gpsimd must stay free for the gather ucode
            ident = consts.tile([128, 128], F32, tag="ident")
            nc.scalar.dma_start(out=ident[:], in_=ident_d[:])
            biasp = consts.tile([128, 64], F32, tag="biasp")
            nc.scalar.dma_start(out=biasp[:], in_=bias_d[:])

            # mixed rawT k-tile: [0:64] = memT shard, [64:128] = GsT (walk sums)
            mixed = consts.tile([128, R], BF16, tag="mixed")
            nc.scalar.dma_start(out=mixed[0:MD, :], in_=memT_d[:])

            # constant tiles; their DMAs are emitted inside the h-loop so
            # they queue behind the od/w1 head flood (needed ~55us onward)
            w1m_sb = consts.tile([128, HT * 128], BF16, tag="w1m")
            w2t_sb = consts.tile([128, HT * MSG], BF16, tag="w2t")
            wh_sb = consts.tile([MD, 3 * MD], BF16, tag="wh")
            wi2_sb = consts.tile([128, 3 * MD], BF16, tag="wi2")
            wp1_sb = consts.tile([MD, 128], BF16, tag="wp1x")
            wp2_sb = consts.tile([128, N], BF16, tag="wp2d")

            # preload the sigmoid/tanh ACT table while the head is DMA-paced
            # (otherwise a 1.28us ACT_TABLE_LOAD lands on the GRU chain)
            warm = gates.tile([MD, 4], F32, tag="warm")
            nc.scalar.activation(warm[:, 0:2], biasp[0:MD, 0:2], AF.Sigmoid)
            nc.scalar.activation(warm[:, 2:4], biasp[0:MD, 0:2], AF.Tanh)

            # walk sums on DVE (gated on the gather), one per node-tile
            m1s = [None] * NT
            for t in range(NT):
                ga3 = gare[:, t * WL * MD:(t + 1) * WL * MD].rearrange(
                    "p (j d) -> p j d", j=WL)
                m4 = gp.tile([128, 4 * MD], F32, tag="m4")
                m43 = m4[:].rearrange("p (j d) -> p j d", j=4)
                nc.vector.tensor_add(out=m43, in0=ga3[:, 0:4, :], in1=ga3[:, 4:8, :])
                m2 = gp.tile([128, 2 * MD], F32, tag="m2")
                m23 = m2[:].rearrange("p (j d) -> p j d", j=2)
                nc.vector.tensor_add(out=m23, in0=m43[:, 0:2, :], in1=m43[:, 2:4, :])
                m1t = gp.tile([128, MD], F32, tag=f"m1_{t}")
                nc.vector.tensor_add(out=m1t[:], in0=m2[:, 0:MD],
                                     in1=m2[:, MD:2 * MD])
                m1s[t] = m1t

            mixed_r = mixed[:]

            # ---- layer 1 (fp8 DoubleRow); the gather (4x ~8.6us Q7 ucode)
            # only completes ~44us in, so h < SPILL spill their od partial to
            # SBUF (freeing the PSUM bank) and replay one per slot once the
            # mixed tile exists; h >= SPILL run a lag-2 finalize.
            SPILL = 9
            psL2 = pacc.tile([128, R], F32, tag="l2")
            odp = consts.tile([128, SPILL * R], F32, tag="odp")
            identx = consts.tile([128, 128], F32, tag="identx")
            pss = {}
            hids = {}
            l2n = [0, 0]
            L2N = [9, 8]   # even/odd L2 stream lengths

            def emit_l2(h):
                half = h % 2
                nc.tensor.matmul(
                    out=psL2[half * 64:(half + 1) * 64, :],
                    lhsT=w2t_sb[:, h * MSG:(h + 1) * MSG],
                    rhs=hids.pop(h)[:],
                    start=(l2n[half] == 0), stop=(l2n[half] == L2N[half] - 1),
                )
                l2n[half] += 1

            def emit_relu(h, src):
                # alternate relu between scalar ACT and vector (add-bias,
                # max 0) so neither engine paces the finalize/replay chain
                hid = hp.tile([128, R], BF16, tag="hid")
                if h % 2 == 0:
                    nc.scalar.activation(hid[:], src, AF.Relu,
                                         bias=biasp[:, h:h + 1])
                else:
                    nc.vector.tensor_scalar(
                        out=hid[:], in0=src, scalar1=biasp[:, h:h + 1],
                        scalar2=0.0, op0=mybir.AluOpType.add,
                        op1=mybir.AluOpType.max)
                hids[h] = hid

            def finalize(h):
                ps = pss.pop(h)
                nc.tensor.matmul(
                    out=ps[:], lhsT=w1m_sb[:, h * 128:(h + 1) * 128],
                    rhs=mixed_r, start=False, stop=True,
                )
                emit_relu(h, ps[:])
                emit_l2(h)

            def replay(h):
                ps = pmm.tile([128, R], F32, tag="mm")
                nc.tensor.matmul(
                    out=ps[:], lhsT=w1m_sb[:, h * 128:(h + 1) * 128],
                    rhs=mixed_r, start=True, stop=True,
                )
                pre = gp.tile([128, R], F32, tag="clt")
                nc.vector.tensor_add(out=pre[:], in0=ps[:],
                                     in1=odp[:, h * R:(h + 1) * R])
                # vector already carries the add: replay relu goes to scalar
                hid = hp.tile([128, R], BF16, tag="hid")
                nc.scalar.activation(hid[:], pre[:], AF.Relu,
                                     bias=biasp[:, h:h + 1])
                hids[h] = hid
                emit_l2(h)

            for h in range(HT):
                if h >= 3:
                    w1tA[h] = w1p.tile([128, HK], FP8, tag="w1tA",
                                       name=f"w1tA{h}")
                    w1tB[h] = w1p.tile([128, HK], FP8, tag="w1tB",
                                       name=f"w1tB{h}")
                    engA = nc.sync if h % 2 == 1 else nc.scalar
                    engB = nc.scalar if h % 2 == 1 else nc.sync
                    engA.dma_start(out=w1tA[h][:], in_=w1h_d[h][:, :HK])
                    engB.dma_start(out=w1tB[h][:], in_=w1h_d[h][:, HK:])
                ps = pmm.tile([128, R], F32, tag="mm")
                for k in range(16):
                    wt = w1tA[h] if k < 8 else w1tB[h]
                    nc.tensor.matmul(
                        out=ps[:],
                        lhsT=wt[:, (k % 8) * 256:(k % 8 + 1) * 256].rearrange(
                            "p (two m) -> p two m", two=2),
                        rhs=odc[k // 2][:, (k % 2) * 2 * R:(k % 2 + 1) * 2 * R]
                        .rearrange("p (two n) -> p two n", two=2),
                        start=(k == 0), stop=(h < SPILL),
                        perf_mode=DR,
                    )
                if h < SPILL:
                    # evacuate pre-activation od partial (scalar; the vector
                    # stream is held by the gather-gated walk sums)
                    nc.scalar.activation(odp[:, h * R:(h + 1) * R], ps[:],
                                         AF.Identity)
                else:
                    pss[h] = ps
                if h == SPILL - 1:
                    # identx = ident + 0*odp[h]: a scheduler pin — the mix
                    # transposes read identx, so no schedule can hoist them
                    # (and their semaphore stalls) into the early od stream
                    tmpid = gp.tile([128, 128], F32, tag="tmpid")
                    nc.vector.tensor_scalar_mul(
                        out=tmpid[:], in0=odp[:, h * R:h * R + 128],
                        scalar1=0.0)
                    nc.vector.tensor_add(out=identx[:], in0=ident[:],
                                         in1=tmpid[:])
                if h == 5:
                    # replays (h<9) read the low half first
                    nc.scalar.dma_start(out=w1m_sb[:, :9 * 128],
                                        in_=w1m_d[:, :9 * 128])
                if h == 6:
                    nc.sync.dma_start(out=w2t_sb[:], in_=w2t_d[:])
                if h == 7:
                    nc.scalar.dma_start(out=w1m_sb[:, 9 * 128:],
                                        in_=w1m_d[:, 9 * 128:])
                if h == 12:
                    # tail-only constants: late so they never delay the w1
                    # stream that gates the od matmuls
                    nc.scalar.dma_start(out=wh_sb[:], in_=wh_d[:])
                    nc.scalar.dma_start(out=wi2_sb[:], in_=wi2_d[:])
                    nc.scalar.dma_start(out=wp1_sb[:], in_=wp1_d[:])
                    nc.sync.dma_start(out=wp2_sb[:], in_=wp2_d[:])
                if h == 14:
                    # h_n = memT @ Wh_n + bias depends only on memT; do it in
                    # the ramp where the PE has slack
                    ps_hn = pmm.tile([MD, R], F32, tag="mm")
                    nc.tensor.matmul(out=ps_hn[:], lhsT=wh_sb[:, 128:192],
                                     rhs=mixed[0:MD, :], start=True, stop=True)
                    hnb = gates.tile([MD, R], F32, tag="hnb")
                    nc.vector.tensor_scalar_add(out=hnb[:], in0=ps_hn[:],
                                                scalar1=biasp[0:MD, 20:21])
                if h == SPILL + 1:
                    # GsT transposes; PE reaches them ~62us, sums done ~60us
                    for t in range(NT):
                        tr = ptr.tile([MD, 128], F32, tag="tr")
                        nc.tensor.transpose(out=tr[:], in_=m1s[t][:],
                                            identity=identx[:])
                        nc.vector.tensor_copy(
                            out=mixed[MD:128, t * 128:(t + 1) * 128], in_=tr[:])
                if h >= SPILL + LAG:
                    finalize(h - LAG)
                    replay(h - SPILL - LAG)
            finalize(HT - 2)
            finalize(HT - 1)
            for h in range(min(HT - SPILL - LAG, SPILL), SPILL):
                replay(h)

            # msg (pre-b2, which is folded into the GRU input bias):
            # both L2 col halves evacuated in one op; the GRU adds them by
            # using duplicated Wi rows (K=128 costs the same as K=64)
            msg2x = gates.tile([128, R], BF16, tag="msg2x")
            nc.scalar.activation(msg2x[:, 0:R // 2], psL2[:, 0:R // 2],
                                 AF.Identity)
            nc.vector.tensor_copy(out=msg2x[:, R // 2:], in_=psL2[:, R // 2:])
            msg_r = msg2x[:]
            memT_r = mixed[0:MD, :]

            # ---- GRU + prediction, column-split so the serial ACT/DVE chain
            #      pipelines across halves and the PE never idles >3.4us.
            ps_r = pmm.tile([MD, R], F32, tag="mm")
            nc.tensor.matmul(out=ps_r[:], lhsT=wi2_sb[:, 0:MD], rhs=msg_r,
                             start=True, stop=False)
            nc.tensor.matmul(out=ps_r[:], lhsT=wh_sb[:, 0:MD], rhs=memT_r,
                             start=False, stop=True)
            ps_z = pmm.tile([MD, R], F32, tag="mm")
            nc.tensor.matmul(out=ps_z[:], lhsT=wi2_sb[:, MD:128], rhs=msg_r,
                             start=True, stop=False)
            nc.tensor.matmul(out=ps_z[:], lhsT=wh_sb[:, MD:128], rhs=memT_r,
                             start=False, stop=True)
            ps_in = pmm.tile([MD, R], F32, tag="mm")
            nc.tensor.matmul(out=ps_in[:], lhsT=wi2_sb[:, 128:192], rhs=msg_r,
                             start=True, stop=True)
            r_t = gates.tile([MD, R], F32, tag="r_t")
            z_t = gates.tile([MD, R], F32, tag="z_t")
            rhn = gates.tile([MD, R], F32, tag="rhn")
            npre = gates.tile([MD, R], F32, tag="npre")
            n_t = gates.tile([MD, R], F32, tag="n_t")
            zc_t = gates.tile([MD, R], F32, tag="zc_t")
            zm_t = gates.tile([MD, R], BF16, tag="zm_t")
            ncz = gates.tile([MD, R], BF16, tag="ncz")
            ps_pred = pacc.tile([128, R], F32, tag="pred")
            act2 = gates.tile([128, R], BF16, tag="act2")
            HR = R // 2
            for x in range(2):
                cs = slice(x * HR, (x + 1) * HR)
                nc.scalar.activation(r_t[:, cs], ps_r[:, cs], AF.Sigmoid,
                                     bias=biasp[0:MD, 17:18])
                nc.scalar.activation(z_t[:, cs], ps_z[:, cs], AF.Sigmoid,
                                     bias=biasp[0:MD, 18:19])
                # upd = (1-z)*n + z*mem = zc*n + zm; zc/zm go on vector right
                # after the z sigmoid (gpsimd would pay a Q7 ucode lib swap
                # after the gathers that serializes the whole chain)
                nc.vector.tensor_scalar(out=zc_t[:, cs], in0=z_t[:, cs],
                                        scalar1=-1.0, scalar2=1.0,
                                        op0=mybir.AluOpType.mult,
                                        op1=mybir.AluOpType.add)
                nc.vector.tensor_mul(out=zm_t[:, cs], in0=z_t[:, cs],
                                     in1=memT_r[:, cs])
                # upd = ncz + zm is absorbed into wp1 by linearity:
                # Wp1@(ncz+zm) = Wp1@ncz + Wp1@zm. The zm part runs right
                # after the z sigmoid, off the r->tanh critical chain
                nc.tensor.matmul(out=ps_pred[:, cs], lhsT=wp1_sb[:],
                                 rhs=zm_t[:, cs], start=True, stop=False)
                nc.vector.tensor_mul(out=rhn[:, cs], in0=r_t[:, cs], in1=hnb[:, cs])
                nc.vector.tensor_add(out=npre[:, cs], in0=ps_in[:, cs], in1=rhn[:, cs])
                nc.scalar.activation(n_t[:, cs], npre[:, cs], AF.Tanh,
                                     bias=biasp[0:MD, 19:20])
                nc.vector.tensor_mul(out=ncz[:, cs], in0=zc_t[:, cs], in1=n_t[:, cs])
                # [Wp1|Wp1] -> act duplicated on partitions 0:64 / 64:128 so
                # pred pairs can row-tile
                nc.tensor.matmul(out=ps_pred[:, cs], lhsT=wp1_sb[:], rhs=ncz[:, cs],
                                 start=False, stop=True)
                nc.scalar.activation(act2[:, cs], ps_pred[:, cs], AF.Relu,
                                     bias=biasp[:, 21:22])

            # ---- prediction m-loop: 16 row-tiled PAIRS (tile_position
            #      (0,0)/(64,0), concurrent on the PE); evacuations alternate
            #      scalar/vector; output staged bf16, 0.5MB DMAs
            GRP = 4
            for m2 in range(16):
                m0, m1 = 2 * m2, 2 * m2 + 1
                psA = pmm.tile([128, R], F32, tag="mm")
                psB = pmm.tile([128, R], F32, tag="mm")
                nc.tensor.matmul(out=psA[:],
                                 lhsT=wp2_sb[0:64, m0 * 128:(m0 + 1) * 128],
                                 rhs=act2[0:64, :], start=True, stop=True)
                nc.tensor.matmul(out=psB[:],
                                 lhsT=wp2_sb[64:128, m1 * 128:(m1 + 1) * 128],
                                 rhs=act2[64:128, :], start=True, stop=True)
                if m0 % GRP == 0:
                    stage = ostg.tile([128, GRP * R], BF16, tag="stage")
                slA = stage[:, (m0 % GRP) * R:(m0 % GRP + 1) * R]
                slB = stage[:, (m1 % GRP) * R:(m1 % GRP + 1) * R]
                nc.scalar.activation(slA, psA[:], AF.Identity,
                                     bias=biasp[:, 22 + m0:23 + m0])
                nc.vector.tensor_scalar_add(out=slB, in0=psB[:],
                                            scalar1=biasp[:, 22 + m1:23 + m1])
                if m1 % GRP == GRP - 1:
                    g = m1 // GRP
                    if g >= 6:
                        # the last transfers gate the end-of-kernel drain:
                        # split them in halves across both free queues
                        st3 = stage[:].rearrange("p (g n) -> p g n", g=GRP)
                        nc.sync.dma_start(
                            out=out_d[:, g * GRP:g * GRP + 2, :],
                            in_=st3[:, 0:2])
                        nc.gpsimd.dma_start(
                            out=out_d[:, g * GRP + 2:(g + 1) * GRP, :],
                            in_=st3[:, 2:4])
                    else:
                        oeng = nc.sync if g % 2 == 0 else nc.gpsimd
                        oeng.dma_start(
                            out=out_d[:, g * GRP:(g + 1) * GRP, :],
                            in_=stage[:].rearrange("p (g n) -> p g n", g=GRP))

    nc.compile()
    return nc


def _get_program():
    global _PROG
    if _PROG is None:
        _PROG = _build_program()
    return _PROG


def _host_prep(memory, od_mat, walks, W_rw, b_rw, W1, b1, W2, b2,
               gru_Wi, gru_bi, gru_Wh, gru_bh, Wp1, bp1, Wp2, bp2):
    import ml_dtypes
    f = np.float32
    bf = ml_dtypes.bfloat16
    e4 = ml_dtypes.float8_e4m3fn
    memory = np.ascontiguousarray(np.asarray(memory), dtype=f)
    od_mat = np.asarray(od_mat)
    walks = np.asarray(walks).astype(np.int32)
    W_rw = np.asarray(W_rw, dtype=f); b_rw = np.asarray(b_rw, dtype=f)
    W1 = np.asarray(W1, dtype=f); b1 = np.asarray(b1, dtype=f)
    W2 = np.asarray(W2, dtype=f); b2 = np.asarray(b2, dtype=f)
    gru_Wi = np.asarray(gru_Wi, dtype=f); gru_bi = np.asarray(gru_bi, dtype=f)
    gru_Wh = np.asarray(gru_Wh, dtype=f); gru_bh = np.asarray(gru_bh, dtype=f)
    Wp1 = np.asarray(Wp1, dtype=f); bp1 = np.asarray(bp1, dtype=f)
    Wp2 = np.asarray(Wp2, dtype=f); bp2 = np.asarray(bp2, dtype=f)

    # layer-1 weights, column-permuted to [od | dest | walk] with W_rw and the
    # 1/8 mean folded into the walk block; HID padded to 2176; whole block
    # scaled x16 so the fp8 od weights sit in e4m3 normal range (1/16 folded
    # into W2; exact since relu(16x)=16relu(x))
    W1od = W1[:, MD:MD + N]
    W1dest = W1[:, 0:MD]
    W1rw = W1[:, MD + N:]
    W1g = (W1rw @ W_rw) / np.float32(8.0)
    W1p = np.concatenate([W1od, W1dest, W1g], axis=1) * np.float32(WSCALE)
    W1pT = np.zeros((33 * 128, HIDP), dtype=f)
    W1pT[:, :HID] = W1p.T
    # w1h[h][p, k*128+c] = W1pT[k*128+p, h*128+c] for the 32 od k-tiles
    # (pairs of adjacent k-tiles feed one DoubleRow matmul);
    # the mixed k-tile (rows 4096:4224) is its own resident tensor w1m
    w1h = np.ascontiguousarray(
        W1pT[:32 * 128].reshape(32, 128, HT, 128)
        .transpose(2, 1, 0, 3).reshape(HT, 128, 32 * 128).astype(e4))
    w1m = np.ascontiguousarray(W1pT[32 * 128:].astype(bf))  # [128, 2176]

    b1p = np.zeros(HIDP, dtype=f)
    b1p[:HID] = (b1 + W1rw @ b_rw) * np.float32(WSCALE)

    W2tp = np.zeros((HIDP, MSG), dtype=f)
    W2tp[:HID] = W2.T / np.float32(WSCALE)
    # w2t[p, h*64+c] = W2tp[h*128+p, c]
    w2t = np.ascontiguousarray(
        W2tp.reshape(HT, 128, MSG).transpose(1, 0, 2).reshape(128, HT * MSG)
        .astype(bf))

    def pad128(v):
        o = np.zeros(128, dtype=f)
        o[:v.shape[0]] = v
        return o

    # b2 folded through the GRU input weights: gi = Wi@(msg'+b2)+bi
    gbi_f = gru_bi + gru_Wi @ b2

    # biases packed as [128 partitions, 64 columns]
    biases = np.zeros((64, 128), dtype=f)
    biases[0:HT] = b1p.reshape(HT, 128)
    grz = gbi_f[:128] + gru_bh[:128]
    biases[17] = pad128(grz[:64])      # r gate bias
    biases[18] = pad128(grz[64:])      # z gate bias
    biases[19] = pad128(gbi_f[128:])
    biases[20] = pad128(gru_bh[128:])
    biases[21] = np.concatenate([bp1, bp1])  # duplicated for act2 row-tiling
    biases[22:54] = bp2.reshape(32, 128)
    biases = np.ascontiguousarray(biases.T)                    # [128, 64]

    WiT = np.ascontiguousarray(gru_Wi.T)                       # [64, 192]
    shared = {
        "mem": memory,
        "w1h": w1h,
        "w1m": w1m,
        "w2t": w2t,
        # Wi rows duplicated: gi = [Wi;Wi] @ [msgA;msgB] (K=128)
        "wi2": np.ascontiguousarray(
            np.concatenate([WiT, WiT], axis=0).astype(bf)),    # [128, 192]
        "wh": np.ascontiguousarray(gru_Wh.T.astype(bf)),       # [64, 192]
        # [Wp1|Wp1]: act lands duplicated on partitions 0:64/64:128
        "wp1x": np.ascontiguousarray(
            np.concatenate([Wp1.T, Wp1.T], axis=1).astype(bf)),  # [64, 128]
        # Wp2T duplicated on partitions 0:64/64:128 for row-tiled pairs
        "wp2d": np.ascontiguousarray(
            np.concatenate([Wp2.T, Wp2.T], axis=0).astype(bf)),  # [128, 4096]
        "biases": biases,
        "ident": np.eye(128, dtype=f),
    }
    in_maps = []
    for c in range(NC):
        sl = slice(c * R, (c + 1) * R)
        odc_np = np.asarray(od_mat[sl], dtype=f)
        # odv[p, k*R+n] = od[c*R+n, k*128+p]
        odv = np.ascontiguousarray(
            odc_np.T.reshape(32, 128, R).transpose(1, 0, 2).reshape(128, 32 * R)
            .astype(e4))
        if USE_DMA_GATHER:
            # dma_gather idx layout: idx[(t*WL+j)*128+p] = walks[t*128+p, j],
            # wrap-16, replicated across the 8 Q7 core stripes
            wkc = walks[sl].reshape(NT, 128, WL).transpose(0, 2, 1).reshape(-1)
            widx = np.ascontiguousarray(np.tile(
                wkc.reshape(-1, 16).T.astype(np.int16), (8, 1)))  # [128, 256]
        else:
            # widx[p, t*WL+j] = walks[c*R + t*128 + p, j]
            widx = np.ascontiguousarray(
                walks[sl].reshape(NT, 128, WL).transpose(1, 0, 2)
                .reshape(128, NT * WL))
        memT = np.ascontiguousarray(memory[sl].T)
        in_maps.append(dict(
            shared,
            memT=np.ascontiguousarray(memT.astype(bf)),
            odv=odv,
            widx=widx,
        ))
    return in_maps


def _assemble(results):
    od = np.empty((N, N), dtype=np.float32)
    for c in range(NC):
        # outm[p, m, n] = od[c*R+n, m*128+p]
        od[c * R:(c + 1) * R, :] = (
            results[c]["outm"].astype(np.float32).transpose(2, 1, 0).reshape(R, N))
    return od


def _install_ntff_shim():
    """The agent image's antenv lacks axon_hooks, so trace=True dies on
    import. Recreate the module with the ctypes-based NTFF hook that
    trn_agent_boot would have registered."""
    import sys
    import types
    if "antenv.axon_hooks" in sys.modules:
        return
    from trn_agent_boot.trn_boot import _ntff_profile_via_ctypes
    hook = _ntff_profile_via_ctypes("/opt/axon/libaxon_pjrt.so")
    mod = types.ModuleType("antenv.axon_hooks")
    mod._hook = hook
    mod.get_axon_ntff_profile_hook = lambda: mod._hook
    mod.set_axon_ntff_profile_hook = lambda h: setattr(mod, "_hook", h)
    sys.modules["antenv.axon_hooks"] = mod


def run(inputs, trace=False):
    """Run on 8 NeuronCores; returns (od [N,N] f32, BassKernelResults)."""
    from concourse.bass_utils import run_bass_kernel_spmd
    if trace:
        try:
            _install_ntff_shim()
        except Exception as e:
            print(f"ntff shim failed ({e}); running without trace")
            trace = False
    nc = _get_program()
    in_maps = _host_prep(**inputs)
    res = run_bass_kernel_spmd(nc, in_maps, list(range(NC)), trace=trace)
    return _assemble(res.results), res


def kernel(**inputs):
    od, _ = run(inputs)
    return od


# revision 69
# speedup vs baseline: 1.1805x; 1.0146x over previous
"""Trainium2 Bass kernel for nn_DiscreteModel (GNN message passing).

Strategy: shard by node rows across 8 cores (512 rows each). All per-node
tensors are kept feature-major ([feature, node]) on-chip so the contraction
dim of every matmul sits on SBUF partitions. The host pre-transposes the
od_mat shard and all weights, folds the random-walk projection W_rw and the
1/8 mean into the layer-1 weight block, and pads HID 2112 -> 2176.

v3 (~125us, from the 158us v2). The PE issues a warm fp8-DoubleRow matmul
every ~216ns (2.4GHz; the 380ns trace "duration" is issue-to-drain), so the
od x W1 block floors at ~59us and everything else must hide behind it:
  head   : od loaded as 8 chunk tiles (2KB partition lines; per-chunk
           matmul gating, odc1 queued ahead of w1tB0 -> first DR matmul
           ~8.4us); w1 h-tiles split in halves across the sync/scalar
           queues; the w1m/w2t constant loads are split across slots
           5/6/7 so their backlog stays under the per-slot DMA slack;
           4 zero-DR warmup matmuls fill the PE until od chunk 0 lands.
  gather : 32 indirect DMAs (~1.1us of gpsimd SWDGE ucode each, ends
           ~56us; dma_gather would pay a ~13us Q7 ucode lib load first).
           gpsimd carries NOTHING else until the output DMAs.
  spill  : h<9 evacuate their od partial to SBUF (scalar ACT) and replay
           after the mixed k-tile exists (lag-2 finalize for h>=9, one
           replay per slot from slot 11, short drain; transposes emitted
           at slot 10, after the walk sums land ~60us). The walk-sum
           transposes read identx = ident + 0*odp[8] -- a data-dep pin so
           the Tile scheduler (sim-driven, reorders freely) cannot hoist
           them and their semaphore stalls into the early od stream.
  L2     : col-tiled pairs (even h -> psum[0:64], odd h -> [64:128],
           concurrent); halves combined for free via duplicated GRU Wi
           rows (gi = [Wi;Wi] @ [msgA;msgB], K=128 costs the same).
  relu   : alternates scalar ACT / vector add+max so neither engine paces
           the finalize chain; replay adds on vector, replay relu scalar.
  tail   : GRU memory operand in bf16 from the resident memT tile; wp1
           output M=128 with [Wp1|Wp1] so act lands duplicated on
           partitions 0:64/64:128; pred runs as 16 row-tiled PAIRS
           (tile_position (0,0)/(64,0), concurrent); evacuations alternate
           scalar/vector into bf16 staging; output DMA per 4 m-tiles on
           sync/gpsimd with [128, 32, R] DRAM layout (4KB lines).
fp8    : the od x W1 block (K=4096 of 4224) runs in fp8e4 DoubleRow mode.
         W1od is scaled x16 on host (relu(16x)=16relu(x); 1/16 folded
         into W2). Rel err ~1.24e-2 vs the 2e-2 gate.
Note: the axon TRN2 fleet drifts run-to-run (same NEFF 124..151us);
compare kernels only back-to-back within one window, min-of-3.
"""

import numpy as np

import concourse.bass as bass
import concourse.bacc as bacc
import concourse.tile as tile
from concourse import mybir

N = 4096        # nodes
MD = 64         # memory dim
MSG = 64        # message dim
WL = 8          # walk length
HID = 2112
HT = 17         # h-tiles (HID padded to 17*128 = 2176)
HIDP = HT * 128
NC = 8          # cores
R = N // NC     # rows (nodes) per core = 512
NT = R // 128   # node tiles per core = 4
F32 = mybir.dt.float32
F32R = mybir.dt.float32r
BF16 = mybir.dt.bfloat16
FP8 = mybir.dt.float8e4
I16 = mybir.dt.int16
I32 = mybir.dt.int32
WSCALE = 16.0   # W1 block scale so fp8 weights sit in e4m3 normal range
USE_DMA_GATHER = False   # Q7 SWDGE gather pays ~13us ucode lib load; the 32
                         # indirect DMAs (1.1us gpsimd ucode each) end sooner
N_WARM = 4      # zero DR matmuls fill the PE only until the first od
                # chunk lands (~8.7us); more would delay the real stream
LAG = 2         # h-tiles between od part and mixed finalize

_PROG = None


def _build_program():
    nc = bacc.Bacc("TRN2", target_bir_lowering=False, debug=False, num_devices=NC)

    # ---- DRAM I/O (all pre-laid-out on host, partition-major) ----
    mem_d = nc.dram_tensor("mem", [N, MD], F32, kind="ExternalInput").ap()
    memT_d = nc.dram_tensor("memT", [MD, R], BF16, kind="ExternalInput").ap()
    od_d = nc.dram_tensor("odv", [128, 32 * R], FP8, kind="ExternalInput").ap()
    if USE_DMA_GATHER:
        widx_d = nc.dram_tensor("widx", [128, NT * WL * 128 // 16], I16,
                                kind="ExternalInput").ap()
    else:
        widx32_d = nc.dram_tensor("widx", [128, NT * WL], I32,
                                  kind="ExternalInput").ap()
    w1h_d = nc.dram_tensor("w1h", [HT, 128, 32 * 128], FP8, kind="ExternalInput").ap()
    w1m_d = nc.dram_tensor("w1m", [128, HT * 128], BF16, kind="ExternalInput").ap()
    w2t_d = nc.dram_tensor("w2t", [128, HT * MSG], BF16, kind="ExternalInput").ap()
    wi2_d = nc.dram_tensor("wi2", [128, 3 * MD], BF16, kind="ExternalInput").ap()
    wh_d = nc.dram_tensor("wh", [MD, 3 * MD], BF16, kind="ExternalInput").ap()
    wp1_d = nc.dram_tensor("wp1x", [MD, 128], BF16, kind="ExternalInput").ap()
    wp2_d = nc.dram_tensor("wp2d", [128, N], BF16, kind="ExternalInput").ap()
    bias_d = nc.dram_tensor("biases", [128, 64], F32, kind="ExternalInput").ap()
    ident_d = nc.dram_tensor("ident", [128, 128], F32, kind="ExternalInput").ap()
    out_d = nc.dram_tensor("outm", [128, 32, R], BF16, kind="ExternalOutput").ap()

    AF = mybir.ActivationFunctionType
    DR = mybir.MatmulPerfMode.DoubleRow
    HK = 8 * 256            # half of a w1 h-tile (k-pairs 0..7)

    with tile.TileContext(nc) as tc:
        with (
            tc.tile_pool(name="consts", bufs=1) as consts,
            tc.tile_pool(name="w1p", bufs=3) as w1p,
            tc.tile_pool(name="gp", bufs=2) as gp,
            tc.tile_pool(name="hp", bufs=4) as hp,
            tc.tile_pool(name="gates", bufs=1) as gates,
            tc.tile_pool(name="ostg", bufs=4) as ostg,
            tc.tile_pool(name="pmm", bufs=5, space="PSUM") as pmm,
            tc.tile_pool(name="pacc", bufs=1, space="PSUM") as pacc,
            tc.tile_pool(name="ptr", bufs=1, space="PSUM") as ptr,
        ):
            # ---- walk indices first: gather feeds the mixed k-tile.
            # Split per node-tile so the first indirect DMA starts as soon
            # as its own 4KB of indices lands.
            if USE_DMA_GATHER:
                wk = consts.tile([128, NT * WL * 128 // 16], I16, tag="wk")
                nc.gpsimd.dma_start(out=wk[:], in_=widx_d[:])
            else:
                wk = consts.tile([128, NT * WL], I32, tag="wk")
                for t in range(NT):
                    nc.gpsimd.dma_start(out=wk[:, t * WL:(t + 1) * WL],
                                        in_=widx32_d[:, t * WL:(t + 1) * WL])

            # head DMA: the first DR matmul needs w1 h0 front half + od chunk
            # 0 only -> both lead their queues; od goes in 8 chunks of 2
            # k-pairs (2KB partition lines for full DMA efficiency), even on
            # sync, odd on scalar, so matmul k gates on chunk k//2.
            w1tA = [None] * HT
            w1tB = [None] * HT
            w1tA[0] = w1p.tile([128, HK], FP8, tag="w1tA", name="w1tA0")
            nc.sync.dma_start(out=w1tA[0][:], in_=w1h_d[0][:, :HK])
            odc = []
            for c in range(8):
                t = consts.tile([128, 4 * R], FP8, tag=f"odc{c}",
                                name=f"odc{c}")
                odc.append(t)
            nc.scalar.dma_start(out=odc[0][:], in_=od_d[:, 0:4 * R])
            nc.scalar.dma_start(out=odc[1][:], in_=od_d[:, 4 * R:8 * R])
            w1tB[0] = w1p.tile([128, HK], FP8, tag="w1tB", name="w1tB0")
            nc.scalar.dma_start(out=w1tB[0][:], in_=w1h_d[0][:, HK:])
            for c in range(2, 8):
                eng = nc.sync if c % 2 == 0 else nc.scalar
                eng.dma_start(out=odc[c][:],
                              in_=od_d[:, c * 4 * R:(c + 1) * 4 * R])
            for h in (1, 2):
                w1tA[h] = w1p.tile([128, HK], FP8, tag="w1tA", name=f"w1tA{h}")
                nc.sync.dma_start(out=w1tA[h][:], in_=w1h_d[h][:, :HK])
                w1tB[h] = w1p.tile([128, HK], FP8, tag="w1tB", name=f"w1tB{h}")
                nc.scalar.dma_start(out=w1tB[h][:], in_=w1h_d[h][:, HK:])

            # Q7 SWDGE gather for all 4096 walk rows:
            # gare[p, (t*WL+j)*MD : +MD] = mem[walks[t*128+p, j]]
            gare = consts.tile([128, NT * WL * MD], F32, tag="gare")
            if USE_DMA_GATHER:
                # >=2048 idxs per instruction hangs the Q7 ucode on HW;
                # 4x1024 (one per node-tile) costs ~1.2us SWDGE each
                for t in range(NT):
                    nc.gpsimd.dma_gather(
                        gare[:, t * WL * MD:(t + 1) * WL * MD].rearrange(
                            "p (g d) -> p g d", g=WL),
                        mem_d[:], wk[:, t * WL * 8:(t + 1) * WL * 8],
                        WL * 128, WL * 128, MD,
                    )
            else:
                for t in range(NT):
                    for j in range(WL):
                        o = (t * WL + j) * MD
                        nc.gpsimd.indirect_dma_start(
                            out=gare[:, o:o + MD],
                            out_offset=None,
                            in_=mem_d[:],
                            in_offset=bass.IndirectOffsetOnAxis(
                                ap=wk[:, t * WL + j:t * WL + j + 1], axis=0),
                        )

            # PE warmup: zero DR matmuls from ~6.5us pull HAM to full clock
            # before the real stream starts (idle >3.4us re-throttles)
            zx = consts.tile([128, 2 * R], FP8, tag="zx")
            nc.vector.memset(zx[:], 0)
            pdum = pmm.tile([128, R], F32, tag="mm")
            for _ in range(N_WARM):
                nc.tensor.matmul(
                    out=pdum[:],
                    lhsT=zx[:, 0:256].rearrange("p (two m) -> p two m", two=2),
                    rhs=zx[:].rearrange("p (two n) -> p two n", two=2),
                    start=True, stop=True, perf_mode=DR,
                )

            # identity from DRAM: gpsimd must stay free for the gather ucode
            ident = consts.tile([128, 128], F32, tag="ident")
            nc.scalar.dma_start(out=ident[:], in_=ident_d[:])
            biasp = consts.tile([128, 64], F32, tag="biasp")
            nc.scalar.dma_start(out=biasp[:], in_=bias_d[:])

            # mixed rawT k-tile: [0:64] = memT shard, [64:128] = GsT (walk sums)
            mixed = consts.tile([128, R], BF16, tag="mixed")
            nc.scalar.dma_start(out=mixed[0:MD, :], in_=memT_d[:])

            # constant tiles; their DMAs are emitted inside the h-loop so
            # they queue behind the od/w1 head flood (needed ~55us onward)
            w1m_sb = consts.tile([128, HT * 128], BF16, tag="w1m")
            w2t_sb = consts.tile([128, HT * MSG], BF16, tag="w2t")
            wh_sb = consts.tile([MD, 3 * MD], BF16, tag="wh")
            wi2_sb = consts.tile([128, 3 * MD], BF16, tag="wi2")
            wp1_sb = consts.tile([MD, 128], BF16, tag="wp1x")
            wp2_sb = consts.tile([128, N], BF16, tag="wp2d")

            # preload the sigmoid/tanh ACT table while the head is DMA-paced
            # (otherwise a 1.28us ACT_TABLE_LOAD lands on the GRU chain)
            warm = gates.tile([MD, 4], F32, tag="warm")
            nc.scalar.activation(warm[:, 0:2], biasp[0:MD, 0:2], AF.Sigmoid)
            nc.scalar.activation(warm[:, 2:4], biasp[0:MD, 0:2], AF.Tanh)

            # walk sums on DVE (gated on the gather), one per node-tile
            m1s = [None] * NT
            for t in range(NT):
                ga3 = gare[:, t * WL * MD:(t + 1) * WL * MD].rearrange(
                    "p (j d) -> p j d", j=WL)
                m4 = gp.tile([128, 4 * MD], F32, tag="m4")
                m43 = m4[:].rearrange("p (j d) -> p j d", j=4)
                nc.vector.tensor_add(out=m43, in0=ga3[:, 0:4, :], in1=ga3[:, 4:8, :])
                m2 = gp.tile([128, 2 * MD], F32, tag="m2")
                m23 = m2[:].rearrange("p (j d) -> p j d", j=2)
                nc.vector.tensor_add(out=m23, in0=m43[:, 0:2, :], in1=m43[:, 2:4, :])
                m1t = gp.tile([128, MD], F32, tag=f"m1_{t}")
                nc.vector.tensor_add(out=m1t[:], in0=m2[:, 0:MD],
                                     in1=m2[:, MD:2 * MD])
                m1s[t] = m1t

            mixed_r = mixed[:]

            # ---- layer 1 (fp8 DoubleRow); the gather (4x ~8.6us Q7 ucode)
            # only completes ~44us in, so h < SPILL spill their od partial to
            # SBUF (freeing the PSUM bank) and replay one per slot once the
            # mixed tile exists; h >= SPILL run a lag-2 finalize.
            SPILL = 9
            psL2 = pacc.tile([128, R], F32, tag="l2")
            odp = consts.tile([128, SPILL * R], F32, tag="odp")
            identx = consts.tile([128, 128], F32, tag="identx")
            pss = {}
            hids = {}
            l2n = [0, 0]
            L2N = [9, 8]   # even/odd L2 stream lengths

            def emit_l2(h):
                half = h % 2
                nc.tensor.matmul(
                    out=psL2[half * 64:(half + 1) * 64, :],
                    lhsT=w2t_sb[:, h * MSG:(h + 1) * MSG],
                    rhs=hids.pop(h)[:],
                    start=(l2n[half] == 0), stop=(l2n[half] == L2N[half] - 1),
                )
                l2n[half] += 1

            def emit_relu(h, src):
                # alternate relu between scalar ACT and vector (add-bias,
                # max 0) so neither engine paces the finalize/replay chain
                hid = hp.tile([128, R], BF16, tag="hid")
                if h % 2 == 0:
                    nc.scalar.activation(hid[:], src, AF.Relu,
                                         bias=biasp[:, h:h + 1])
                else:
                    nc.vector.tensor_scalar(
                        out=hid[:], in0=src, scalar1=biasp[:, h:h + 1],
                        scalar2=0.0, op0=mybir.AluOpType.add,
                        op1=mybir.AluOpType.max)
                hids[h] = hid

            def finalize(h):
                ps = pss.pop(h)
                nc.tensor.matmul(
                    out=ps[:], lhsT=w1m_sb[:, h * 128:(h + 1) * 128],
                    rhs=mixed_r, start=False, stop=True,
                )
                emit_relu(h, ps[:])
                emit_l2(h)

            def replay(h):
                ps = pmm.tile([128, R], F32, tag="mm")
                nc.tensor.matmul(
                    out=ps[:], lhsT=w1m_sb[:, h * 128:(h + 1) * 128],
                    rhs=mixed_r, start=True, stop=True,
                )
                pre = gp.tile([128, R], F32, tag="clt")
                nc.vector.tensor_add(out=pre[:], in0=ps[:],
                                     in1=odp[:, h * R:(h + 1) * R])
                # vector already carries the add: replay relu goes to scalar
                hid = hp.tile([128, R], BF16, tag="hid")
                nc.scalar.activation(hid[:], pre[:], AF.Relu,
                                     bias=biasp[:, h:h + 1])
                hids[h] = hid
                emit_l2(h)

            for h in range(HT):
                if h >= 3:
                    w1tA[h] = w1p.tile([128, HK], FP8, tag="w1tA",
                                       name=f"w1tA{h}")
                    w1tB[h] = w1p.tile([128, HK], FP8, tag="w1tB",
                                       name=f"w1tB{h}")
                    engA = nc.sync if h % 2 == 1 else nc.scalar
                    engB = nc.scalar if h % 2 == 1 else nc.sync
                    engA.dma_start(out=w1tA[h][:], in_=w1h_d[h][:, :HK])
                    engB.dma_start(out=w1tB[h][:], in_=w1h_d[h][:, HK:])
                ps = pmm.tile([128, R], F32, tag="mm")
                for k in range(16):
                    wt = w1tA[h] if k < 8 else w1tB[h]
                    nc.tensor.matmul(
                        out=ps[:],
                        lhsT=wt[:, (k % 8) * 256:(k % 8 + 1) * 256].rearrange(
                            "p (two m) -> p two m", two=2),
                        rhs=odc[k // 2][:, (k % 2) * 2 * R:(k % 2 + 1) * 2 * R]
                        .rearrange("p (two n) -> p two n", two=2),
                        start=(k == 0), stop=(h < SPILL),
                        perf_mode=DR,
                    )
                if h < SPILL:
                    # evacuate pre-activation od partial (scalar; the vector
                    # stream is held by the gather-gated walk sums)
                    nc.scalar.activation(odp[:, h * R:(h + 1) * R], ps[:],
                                         AF.Identity)
                else:
                    pss[h] = ps
                if h == SPILL - 1:
                    # identx = ident + 0*odp[h]: a scheduler pin — the mix
                    # transposes read identx, so no schedule can hoist them
                    # (and their semaphore stalls) into the early od stream
                    tmpid = gp.tile([128, 128], F32, tag="tmpid")
                    nc.vector.tensor_scalar_mul(
                        out=tmpid[:], in0=odp[:, h * R:h * R + 128],
                        scalar1=0.0)
                    nc.vector.tensor_add(out=identx[:], in0=ident[:],
                                         in1=tmpid[:])
                if h == 5:
                    # replays (h<9) read the low half first
                    nc.scalar.dma_start(out=w1m_sb[:, :9 * 128],
                                        in_=w1m_d[:, :9 * 128])
                if h == 6:
                    nc.sync.dma_start(out=w2t_sb[:], in_=w2t_d[:])
                if h == 7:
                    nc.scalar.dma_start(out=w1m_sb[:, 9 * 128:],
                                        in_=w1m_d[:, 9 * 128:])
                if h == 12:
                    # tail-only constants: late so they never delay the w1
                    # stream that gates the od matmuls
                    nc.scalar.dma_start(out=wh_sb[:], in_=wh_d[:])
                    nc.scalar.dma_start(out=wi2_sb[:], in_=wi2_d[:])
                    nc.scalar.dma_start(out=wp1_sb[:], in_=wp1_d[:])
                    nc.sync.dma_start(out=wp2_sb[:], in_=wp2_d[:])
                if h == 14:
                    # h_n = memT @ Wh_n + bias depends only on memT; do it in
                    # the ramp where the PE has slack
                    ps_hn = pmm.tile([MD, R], F32, tag="mm")
                    nc.tensor.matmul(out=ps_hn[:], lhsT=wh_sb[:, 128:192],
                                     rhs=mixed[0:MD, :], start=True, stop=True)
                    hnb = gates.tile([MD, R], F32, tag="hnb")
                    nc.vector.tensor_scalar_add(out=hnb[:], in0=ps_hn[:],
                                                scalar1=biasp[0:MD, 20:21])
                if h == SPILL + 1:
                    # GsT transposes; PE reaches them ~62us, sums done ~60us
                    for t in range(NT):
                        tr = ptr.tile([MD, 128], F32, tag="tr")
                        nc.tensor.transpose(out=tr[:], in_=m1s[t][:],
                                            identity=identx[:])
                        nc.vector.tensor_copy(
                            out=mixed[MD:128, t * 128:(t + 1) * 128], in_=tr[:])
                if h >= SPILL + LAG:
                    finalize(h - LAG)
                    replay(h - SPILL - LAG)
            finalize(HT - 2)
            finalize(HT - 1)
            for h in range(min(HT - SPILL - LAG, SPILL), SPILL):
                replay(h)

            # msg (pre-b2, which is folded into the GRU input bias):
            # both L2 col halves evacuated in one op; the GRU adds them by
            # using duplicated Wi rows (K=128 costs the same as K=64)
            msg2x = gates.tile([128, R], BF16, tag="msg2x")
            nc.scalar.activation(msg2x[:, 0:R // 2], psL2[:, 0:R // 2],
                                 AF.Identity)
            nc.vector.tensor_copy(out=msg2x[:, R // 2:], in_=psL2[:, R // 2:])
            msg_r = msg2x[:]
            memT_r = mixed[0:MD, :]

            # ---- GRU + prediction, column-split so the serial ACT/DVE chain
            #      pipelines across halves and the PE never idles >3.4us.
            ps_r = pmm.tile([MD, R], F32, tag="mm")
            nc.tensor.matmul(out=ps_r[:], lhsT=wi2_sb[:, 0:MD], rhs=msg_r,
                             start=True, stop=False)
            nc.tensor.matmul(out=ps_r[:], lhsT=wh_sb[:, 0:MD], rhs=memT_r,
                             start=False, stop=True)
            ps_z = pmm.tile([MD, R], F32, tag="mm")
            nc.tensor.matmul(out=ps_z[:], lhsT=wi2_sb[:, MD:128], rhs=msg_r,
                             start=True, stop=False)
            nc.tensor.matmul(out=ps_z[:], lhsT=wh_sb[:, MD:128], rhs=memT_r,
                             start=False, stop=True)
            ps_in = pmm.tile([MD, R], F32, tag="mm")
            nc.tensor.matmul(out=ps_in[:], lhsT=wi2_sb[:, 128:192], rhs=msg_r,
                             start=True, stop=True)
            r_t = gates.tile([MD, R], F32, tag="r_t")
            z_t = gates.tile([MD, R], F32, tag="z_t")
            rhn = gates.tile([MD, R], F32, tag="rhn")
            npre = gates.tile([MD, R], F32, tag="npre")
            n_t = gates.tile([MD, R], F32, tag="n_t")
            zc_t = gates.tile([MD, R], F32, tag="zc_t")
            zm_t = gates.tile([MD, R], BF16, tag="zm_t")
            ncz = gates.tile([MD, R], BF16, tag="ncz")
            ps_pred = pacc.tile([128, R], F32, tag="pred")
            act2 = gates.tile([128, R], BF16, tag="act2")
            HR = R // 2
            for x in range(2):
                cs = slice(x * HR, (x + 1) * HR)
                nc.scalar.activation(r_t[:, cs], ps_r[:, cs], AF.Sigmoid,
                                     bias=biasp[0:MD, 17:18])
                nc.scalar.activation(z_t[:, cs], ps_z[:, cs], AF.Sigmoid,
                                     bias=biasp[0:MD, 18:19])
                # upd = (1-z)*n + z*mem = zc*n + zm; zc/zm go on vector right
                # after the z sigmoid (gpsimd would pay a Q7 ucode lib swap
                # after the gathers that serializes the whole chain)
                nc.vector.tensor_scalar(out=zc_t[:, cs], in0=z_t[:, cs],
                                        scalar1=-1.0, scalar2=1.0,
                                        op0=mybir.AluOpType.mult,
                                        op1=mybir.AluOpType.add)
                nc.vector.tensor_mul(out=zm_t[:, cs], in0=z_t[:, cs],
                                     in1=memT_r[:, cs])
                # upd = ncz + zm is absorbed into wp1 by linearity:
                # Wp1@(ncz+zm) = Wp1@ncz + Wp1@zm. The zm part runs right
                # after the z sigmoid, off the r->tanh critical chain
                nc.tensor.matmul(out=ps_pred[:, cs], lhsT=wp1_sb[:],
                                 rhs=zm_t[:, cs], start=True, stop=False)
                nc.vector.tensor_mul(out=rhn[:, cs], in0=r_t[:, cs], in1=hnb[:, cs])
                nc.vector.tensor_add(out=npre[:, cs], in0=ps_in[:, cs], in1=rhn[:, cs])
                nc.scalar.activation(n_t[:, cs], npre[:, cs], AF.Tanh,
                                     bias=biasp[0:MD, 19:20])
                nc.vector.tensor_mul(out=ncz[:, cs], in0=zc_t[:, cs], in1=n_t[:, cs])
                # [Wp1|Wp1] -> act duplicated on partitions 0:64 / 64:128 so
                # pred pairs can row-tile
                nc.tensor.matmul(out=ps_pred[:, cs], lhsT=wp1_sb[:], rhs=ncz[:, cs],
                                 start=False, stop=True)
                # act2 gates the pred pairs: split the relu across scalar
                # and vector so each half lands in ~half the time
                mid = x * HR + HR // 2
                nc.scalar.activation(act2[:, x * HR:mid],
                                     ps_pred[:, x * HR:mid], AF.Relu,
                                     bias=biasp[:, 21:22])
                nc.vector.tensor_scalar(
                    out=act2[:, mid:(x + 1) * HR],
                    in0=ps_pred[:, mid:(x + 1) * HR],
                    scalar1=biasp[:, 21:22], scalar2=0.0,
                    op0=mybir.AluOpType.add, op1=mybir.AluOpType.max)

            # ---- prediction m-loop: 16 row-tiled PAIRS (tile_position
            #      (0,0)/(64,0), concurrent on the PE); evacuations alternate
            #      scalar/vector; output staged bf16, 0.5MB DMAs
            GRP = 4
            for m2 in range(16):
                m0, m1 = 2 * m2, 2 * m2 + 1
                psA = pmm.tile([128, R], F32, tag="mm")
                psB = pmm.tile([128, R], F32, tag="mm")
                nc.tensor.matmul(out=psA[:],
                                 lhsT=wp2_sb[0:64, m0 * 128:(m0 + 1) * 128],
                                 rhs=act2[0:64, :], start=True, stop=True)
                nc.tensor.matmul(out=psB[:],
                                 lhsT=wp2_sb[64:128, m1 * 128:(m1 + 1) * 128],
                                 rhs=act2[64:128, :], start=True, stop=True)
                if m0 % GRP == 0:
                    stage = ostg.tile([128, GRP * R], BF16, tag="stage")
                slA = stage[:, (m0 % GRP) * R:(m0 % GRP + 1) * R]
                slB = stage[:, (m1 % GRP) * R:(m1 % GRP + 1) * R]
                nc.scalar.activation(slA, psA[:], AF.Identity,
                                     bias=biasp[:, 22 + m0:23 + m0])
                nc.vector.tensor_scalar_add(out=slB, in0=psB[:],
                                            scalar1=biasp[:, 22 + m1:23 + m1])
                if m1 % GRP == GRP - 1:
                    g = m1 // GRP
                    if g >= 6:
                        # the last transfers gate the end-of-kernel drain:
                        # split them in halves across both free queues
                        st3 = stage[:].rearrange("p (g n) -> p g n", g=GRP)
                        nc.sync.dma_start(
                            out=out_d[:, g * GRP:g * GRP + 2, :],
                            in_=st3[:, 0:2])
                        nc.gpsimd.dma_start(
                            out=out_d[:, g * GRP + 2:(g + 1) * GRP, :],
                            in_=st3[:, 2:4])
                    else:
                        oeng = nc.sync if g % 2 == 0 else nc.gpsimd
                        oeng.dma_start(
                            out=out_d[:, g * GRP:(g + 1) * GRP, :],
                            in_=stage[:].rearrange("p (g n) -> p g n", g=GRP))

    nc.compile()
    return nc


def _get_program():
    global _PROG
    if _PROG is None:
        _PROG = _build_program()
    return _PROG


def _host_prep(memory, od_mat, walks, W_rw, b_rw, W1, b1, W2, b2,
               gru_Wi, gru_bi, gru_Wh, gru_bh, Wp1, bp1, Wp2, bp2):
    import ml_dtypes
    f = np.float32
    bf = ml_dtypes.bfloat16
    e4 = ml_dtypes.float8_e4m3fn
    memory = np.ascontiguousarray(np.asarray(memory), dtype=f)
    od_mat = np.asarray(od_mat)
    walks = np.asarray(walks).astype(np.int32)
    W_rw = np.asarray(W_rw, dtype=f); b_rw = np.asarray(b_rw, dtype=f)
    W1 = np.asarray(W1, dtype=f); b1 = np.asarray(b1, dtype=f)
    W2 = np.asarray(W2, dtype=f); b2 = np.asarray(b2, dtype=f)
    gru_Wi = np.asarray(gru_Wi, dtype=f); gru_bi = np.asarray(gru_bi, dtype=f)
    gru_Wh = np.asarray(gru_Wh, dtype=f); gru_bh = np.asarray(gru_bh, dtype=f)
    Wp1 = np.asarray(Wp1, dtype=f); bp1 = np.asarray(bp1, dtype=f)
    Wp2 = np.asarray(Wp2, dtype=f); bp2 = np.asarray(bp2, dtype=f)

    # layer-1 weights, column-permuted to [od | dest | walk] with W_rw and the
    # 1/8 mean folded into the walk block; HID padded to 2176; whole block
    # scaled x16 so the fp8 od weights sit in e4m3 normal range (1/16 folded
    # into W2; exact since relu(16x)=16relu(x))
    W1od = W1[:, MD:MD + N]
    W1dest = W1[:, 0:MD]
    W1rw = W1[:, MD + N:]
    W1g = (W1rw @ W_rw) / np.float32(8.0)
    W1p = np.concatenate([W1od, W1dest, W1g], axis=1) * np.float32(WSCALE)
    W1pT = np.zeros((33 * 128, HIDP), dtype=f)
    W1pT[:, :HID] = W1p.T
    # w1h[h][p, k*128+c] = W1pT[k*128+p, h*128+c] for the 32 od k-tiles
    # (pairs of adjacent k-tiles feed one DoubleRow matmul);
    # the mixed k-tile (rows 4096:4224) is its own resident tensor w1m
    w1h = np.ascontiguousarray(
        W1pT[:32 * 128].reshape(32, 128, HT, 128)
        .transpose(2, 1, 0, 3).reshape(HT, 128, 32 * 128).astype(e4))
    w1m = np.ascontiguousarray(W1pT[32 * 128:].astype(bf))  # [128, 2176]

    b1p = np.zeros(HIDP, dtype=f)
    b1p[:HID] = (b1 + W1rw @ b_rw) * np.float32(WSCALE)

    W2tp = np.zeros((HIDP, MSG), dtype=f)
    W2tp[:HID] = W2.T / np.float32(WSCALE)
    # w2t[p, h*64+c] = W2tp[h*128+p, c]
    w2t = np.ascontiguousarray(
        W2tp.reshape(HT, 128, MSG).transpose(1, 0, 2).reshape(128, HT * MSG)
        .astype(bf))

    def pad128(v):
        o = np.zeros(128, dtype=f)
        o[:v.shape[0]] = v
        return o

    # b2 folded through the GRU input weights: gi = Wi@(msg'+b2)+bi
    gbi_f = gru_bi + gru_Wi @ b2

    # biases packed as [128 partitions, 64 columns]
    biases = np.zeros((64, 128), dtype=f)
    biases[0:HT] = b1p.reshape(HT, 128)
    grz = gbi_f[:128] + gru_bh[:128]
    biases[17] = pad128(grz[:64])      # r gate bias
    biases[18] = pad128(grz[64:])      # z gate bias
    biases[19] = pad128(gbi_f[128:])
    biases[20] = pad128(gru_bh[128:])
    biases[21] = np.concatenate([bp1, bp1])  # duplicated for act2 row-tiling
    biases[22:54] = bp2.reshape(32, 128)
    biases = np.ascontiguousarray(biases.T)                    # [128, 64]

    WiT = np.ascontiguousarray(gru_Wi.T)                       # [64, 192]
    shared = {
        "mem": memory,
        "w1h": w1h,
        "w1m": w1m,
        "w2t": w2t,
        # Wi rows duplicated: gi = [Wi;Wi] @ [msgA;msgB] (K=128)
        "wi2": np.ascontiguousarray(
            np.concatenate([WiT, WiT], axis=0).astype(bf)),    # [128, 192]
        "wh": np.ascontiguousarray(gru_Wh.T.astype(bf)),       # [64, 192]
        # [Wp1|Wp1]: act lands duplicated on partitions 0:64/64:128
        "wp1x": np.ascontiguousarray(
            np.concatenate([Wp1.T, Wp1.T], axis=1).astype(bf)),  # [64, 128]
        # Wp2T duplicated on partitions 0:64/64:128 for row-tiled pairs
        "wp2d": np.ascontiguousarray(
            np.concatenate([Wp2.T, Wp2.T], axis=0).astype(bf)),  # [128, 4096]
        "biases": biases,
        "ident": np.eye(128, dtype=f),
    }
    in_maps = []
    for c in range(NC):
        sl = slice(c * R, (c + 1) * R)
        odc_np = np.asarray(od_mat[sl], dtype=f)
        # odv[p, k*R+n] = od[c*R+n, k*128+p]
        odv = np.ascontiguousarray(
            odc_np.T.reshape(32, 128, R).transpose(1, 0, 2).reshape(128, 32 * R)
            .astype(e4))
        if USE_DMA_GATHER:
            # dma_gather idx layout: idx[(t*WL+j)*128+p] = walks[t*128+p, j],
            # wrap-16, replicated across the 8 Q7 core stripes
            wkc = walks[sl].reshape(NT, 128, WL).transpose(0, 2, 1).reshape(-1)
            widx = np.ascontiguousarray(np.tile(
                wkc.reshape(-1, 16).T.astype(np.int16), (8, 1)))  # [128, 256]
        else:
            # widx[p, t*WL+j] = walks[c*R + t*128 + p, j]
            widx = np.ascontiguousarray(
                walks[sl].reshape(NT, 128, WL).transpose(1, 0, 2)
                .reshape(128, NT * WL))
        memT = np.ascontiguousarray(memory[sl].T)
        in_maps.append(dict(
            shared,
            memT=np.ascontiguousarray(memT.astype(bf)),
            odv=odv,
            widx=widx,
        ))
    return in_maps


def _assemble(results):
    od = np.empty((N, N), dtype=np.float32)
    for c in range(NC):
        # outm[p, m, n] = od[c*R+n, m*128+p]
        od[c * R:(c + 1) * R, :] = (
            results[c]["outm"].astype(np.float32).transpose(2, 1, 0).reshape(R, N))
    return od


def _install_ntff_shim():
    """The agent image's antenv lacks axon_hooks, so trace=True dies on
    import. Recreate the module with the ctypes-based NTFF hook that
    trn_agent_boot would have registered."""
    import sys
    import types
    if "antenv.axon_hooks" in sys.modules:
        return
    from trn_agent_boot.trn_boot import _ntff_profile_via_ctypes
    hook = _ntff_profile_via_ctypes("/opt/axon/libaxon_pjrt.so")
    mod = types.ModuleType("antenv.axon_hooks")
    mod._hook = hook
    mod.get_axon_ntff_profile_hook = lambda: mod._hook
    mod.set_axon_ntff_profile_hook = lambda h: setattr(mod, "_hook", h)
    sys.modules["antenv.axon_hooks"] = mod


def run(inputs, trace=False):
    """Run on 8 NeuronCores; returns (od [N,N] f32, BassKernelResults)."""
    from concourse.bass_utils import run_bass_kernel_spmd
    if trace:
        try:
            _install_ntff_shim()
        except Exception as e:
            print(f"ntff shim failed ({e}); running without trace")
            trace = False
    nc = _get_program()
    in_maps = _host_prep(**inputs)
    res = run_bass_kernel_spmd(nc, in_maps, list(range(NC)), trace=trace)
    return _assemble(res.results), res


def kernel(**inputs):
    od, _ = run(inputs)
    return od
